# revision 8
# baseline (speedup 1.0000x reference)
"""Trainium2 Bass kernel: 3-layer GAT (nn_GAT_62182536511748).

Strategy (8 NeuronCores, SPMD, fp16 pair-block gather, v3):
  - Nodes sharded contiguously across cores (6250 valid/core, padded to
    6272 = 49*128). dst == repeat(arange(N), 16): 16 in-edges per node.
  - Per layer each core computes feat = x_shard @ Wext (fp16 PE, PSUM
    fp32) where Wext = [W | W@al | W@ar] also yields el/er. Rows
    [feat|el] are packed into fp16 PAIR blocks and AllGather'd in 4 row
    chunks; the next layer's AllGather chunks are triggered from inside
    the previous layer's gather stream so the CC transfers overlap the
    gather drain.
  - Edge phase: per 128-node group (2048 edges), two 1024-index
    dma_gather instructions (int16 pair indices = src//2, rotating over
    the 4 SWDGE queues) fetch one pair block per edge. The gather drain
    (~410 descriptors/us, descriptor-count-bound) is the wall; the rest
    is spread across engines to hide underneath it:
      DVE: pair-select (copy_predicated), attention logits, softmax
           denominator, normalize + relu.
      Scalar: exp (fp16 out), PSUM->SBUF copies.
      GpSimd: alpha-weighted multiply + first reduction level.
      PE: remaining slot reduction as identity-matmul PSUM accumulation,
          feat matmuls, output transpose into the SBUF-resident hT tile
          that feeds the next layer (no DRAM round trip).
  - Edge math for group g is emitted two groups behind its gathers so
    every engine stream has slack; softmax skips max-subtraction
    (logits are O(1)).
"""

import os
import numpy as np

# ---- fixed problem dims -------------------------------------------------
N = 50000
DEG = 16
IN = 256
HID = 32
HEAD = 4
OUT = 40
HH = HID * HEAD  # 128
NEG_SLOPE = 0.2
NCORES = 8
NV = N // NCORES          # 6250 valid nodes per core
G = 49                    # groups of 128 rows
NS_PAD = G * 128          # 6272
NSB = NS_PAD // 2         # 3136 local pair rows
NBLK = NCORES * NSB       # 25088 global pair rows

# AllGather row chunks (in groups)
GCH = [0, 13, 25, 37, 49]
PCH = [g * 64 for g in GCH]               # local pair-row bounds
PC = [PCH[i + 1] - PCH[i] for i in range(4)]
GB = [NCORES * p for p in PCH]            # global pair-row bases

SUB1, SUB2, SUB3 = HH + HEAD, HH + 1, OUT + 1   # 132, 129, 41
BLK12, BLK3 = 384, 128                    # table pitch (f16 elems)
NIDX = 1024
DEFER = 2

_PROGRAM_CACHE = {}
LAST_RESULTS = None


def _dma_gather_raw(nc, mybir, out_ap, in_ap, idxs_ap, num_idxs, elem_size,
                    elem_step, queue_num=0):
    """dma_gather minus the over-strict elem%256B assert (stride must still
    be a 256B multiple; verified on HW with 528B/516B/164B elems)."""
    eng = nc.gpsimd
    stride_bytes = elem_step * mybir.dt.size(in_ap.dtype)
    assert stride_bytes % 256 == 0 and stride_bytes // 256 < 256
    _in_ap = eng.lower_ap_dma(in_ap, for_custom_bir_dma=True)
    _idxs_ap = eng.lower_ap(idxs_ap)
    _out_ap = eng.lower_ap(out_ap)
    return eng.add_instruction(
        mybir.InstDMAGatherAnt(
            name=nc.get_next_instruction_name(),
            ins=[*_in_ap, _idxs_ap,
                 eng.lower_val_access(eng.to_reg(num_idxs))],
            outs=[_out_ap],
            transpose=False, num_idxs=num_idxs, elem_size=elem_size,
            stride_bytes_256=stride_bytes // 256, gen_mode=0,
            single_packet=True, queue_num=queue_num,
            sbuf_tokens_per_rank=0, sbuf_free_dim_per_rank=0,
            sbuf_free_dim_pad_per_rank=0, sbuf_byte_offset=0,
        ))


# ========================================================================
# device program
# ========================================================================
def _build_program(ncores: int):
    from concourse import bass, mybir, tile, bacc
    from concourse.masks import make_identity
    from concourse.library_config import mlp

    f32 = mybir.dt.float32
    f16 = mybir.dt.float16
    i16 = mybir.dt.int16
    u8 = mybir.dt.uint8
    AX = mybir.AxisListType
    OPT = mybir.AluOpType
    AF = mybir.ActivationFunctionType

    nc = bacc.Bacc(
        "TRN2", target_bir_lowering=False, debug=False,
        enable_asserts=False, num_devices=ncores, num_swdge_queues=4)

    # ---- kernel I/O ----
    x0t_d = nc.dram_tensor("x0t", [IN, NS_PAD], f16, kind="ExternalInput").ap()
    idx_d = nc.dram_tensor("idx", [128, G * 128], i16,
                           kind="ExternalInput").ap()
    sel_d = nc.dram_tensor("sel", [128, G * DEG], u8,
                           kind="ExternalInput").ap()
    w1_d = nc.dram_tensor("w1", [IN, HH + 2 * HEAD], f16,
                          kind="ExternalInput").ap()
    wh_d = nc.dram_tensor("wh", [HH, HH + 2], f16, kind="ExternalInput").ap()
    w2_d = nc.dram_tensor("w2", [HH, OUT + 2], f16,
                          kind="ExternalInput").ap()
    b1_d = nc.dram_tensor("b1", [128, HH], f32, kind="ExternalInput").ap()
    bh_d = nc.dram_tensor("bh", [128, HH], f32, kind="ExternalInput").ap()
    b2_d = nc.dram_tensor("b2", [128, OUT], f32, kind="ExternalInput").ap()
    out_d = nc.dram_tensor("out", [NS_PAD, OUT], f32,
                           kind="ExternalOutput").ap()

    shared = "Shared" if ncores > 4 else "Local"
    gs_t = {}
    for l, blk in ((1, BLK12), (2, BLK12), (3, BLK3)):
        gs_t[l] = [nc.dram_tensor(f"gs{l}_{c}", [PC[c], blk], f16).ap()
                   for c in range(4)]
    gf1_d = nc.dram_tensor("gf1", [NBLK, BLK12], f16, addr_space=shared).ap()
    gf2_d = nc.dram_tensor("gf2", [NBLK, BLK12], f16, addr_space=shared).ap()
    gf3_d = nc.dram_tensor("gf3", [NBLK, BLK3], f16, addr_space=shared).ap()

    rgroups = [list(range(ncores))]

    with tile.TileContext(nc) as tc:
        with (
            tc.tile_pool(name="const", bufs=1) as cp,
            tc.tile_pool(name="feat", bufs=3) as fp,
            tc.tile_pool(name="edge", bufs=3) as ep,
            tc.tile_pool(name="psum", bufs=2, space="PSUM") as pp,
        ):
            nc.gpsimd.load_library(mlp)
            ident = cp.tile([128, 128], f16)
            make_identity(nc, ident[:])
            idx_sb = cp.tile([128, G * 128], i16)
            nc.sync.dma_start(out=idx_sb[:], in_=idx_d[:, :])
            sel_sb = cp.tile([128, G * DEG], u8)
            nc.sync.dma_start(out=sel_sb[:], in_=sel_d[:, :])
            x0a = cp.tile([128, NS_PAD], f16)
            x0b = cp.tile([128, NS_PAD], f16)
            nc.sync.dma_start(out=x0a[:], in_=x0t_d[0:128, :])
            nc.sync.dma_start(out=x0b[:], in_=x0t_d[128:256, :])
            w1a = cp.tile([128, HH + 2 * HEAD], f16)
            w1b = cp.tile([128, HH + 2 * HEAD], f16)
            nc.sync.dma_start(out=w1a[:], in_=w1_d[0:128, :])
            nc.sync.dma_start(out=w1b[:], in_=w1_d[128:256, :])
            wh_sb = cp.tile([128, HH + 2], f16)
            nc.sync.dma_start(out=wh_sb[:], in_=wh_d[:, :])
            w2_sb = cp.tile([128, OUT + 2], f16)
            nc.sync.dma_start(out=w2_sb[:], in_=w2_d[:, :])
            b1_t = cp.tile([128, HH], f32)
            nc.sync.dma_start(out=b1_t[:], in_=b1_d[:, :])
            bh_t = cp.tile([128, HH], f32)
            nc.sync.dma_start(out=bh_t[:], in_=bh_d[:, :])
            b2_t = cp.tile([128, OUT], f32)
            nc.sync.dma_start(out=b2_t[:], in_=b2_d[:, :])
            er1 = cp.tile([128, G * HEAD], f32)
            er2 = cp.tile([128, G], f32)
            er3 = cp.tile([128, G], f32)
            hT1 = cp.tile([128, NS_PAD], f16)
            hT2 = cp.tile([128, NS_PAD], f16)

            def feat_group(lname, g, lhsT_tiles, w_tiles, nw, sub, er_t, H,
                           gs_list):
                s = slice(g * 128, (g + 1) * 128)
                fps = pp.tile([128, HH + 2 * HEAD], f32, tag="fps",
                              name=f"{lname}_fps{g}")
                nchunk = len(lhsT_tiles)
                for c in range(nchunk):
                    nc.tensor.matmul(
                        fps[:, 0:nw], lhsT=lhsT_tiles[c][:, s],
                        rhs=w_tiles[c][:],
                        start=(c == 0), stop=(c == nchunk - 1))
                grow = fp.tile([128, sub], f16, tag=f"grow{lname}",
                               name=f"{lname}_grow{g}")
                nc.scalar.activation(out=grow[:], in_=fps[:, 0:sub],
                                     func=AF.Copy)
                nc.scalar.activation(out=er_t[:, g * H:(g + 1) * H],
                                     in_=fps[:, sub:sub + H], func=AF.Copy)
                c = 0
                while g >= GCH[c + 1]:
                    c += 1
                p0 = g * 64 - PCH[c]
                dst = gs_list[c][p0:p0 + 64, 0:2 * sub].rearrange(
                    "b (s c) -> b s c", c=sub)
                nc.sync.dma_start(out=dst, in_=grow[:])

            def ag_chunk(l, c, gf_ap):
                nc.gpsimd.collective_compute(
                    "AllGather", OPT.bypass, replica_groups=rgroups,
                    ins=[gs_t[l][c][:, :]],
                    outs=[gf_ap[GB[c]:GB[c + 1], :]])

            def emit_gathers(lname, g, gf_ap, blk, sub, bigtag, bigw):
                ELEM = 2 * sub
                big = ep.tile([128, bigw], f16, tag=bigtag, bufs=6,
                              name=f"{lname}_big{g}")
                for h in range(2):
                    _dma_gather_raw(
                        nc, mybir,
                        big[:, h * 8 * ELEM:(h + 1) * 8 * ELEM],
                        gf_ap[:, 0:ELEM],
                        idx_sb[:, g * 128 + h * 64:g * 128 + (h + 1) * 64],
                        NIDX, ELEM, blk, queue_num=(2 * g + h) % 4)
                return big

            def edge_math(lname, g, big, sub, HD, H, b_t, er_t, mode,
                          hT_out):
                D = HD // H
                ELEM = 2 * sub
                bv = big[:, 0:DEG * ELEM].rearrange("p (k r) -> p k r",
                                                    r=ELEM)
                lo = bv[:, :, 0:sub]
                hi = bv[:, :, sub:2 * sub]
                mask = (sel_sb[:, g * DEG:(g + 1) * DEG]
                        .unsqueeze(2).to_broadcast((128, DEG, sub)))
                nc.vector.copy_predicated(out=lo, mask=mask, data=hi)
                feat_e = bv[:, :, 0:HD]
                el_e = bv[:, :, HD:HD + H]
                # e = el + er  (er broadcast along slots)
                e_t = ep.tile([128, DEG * H], f32, tag="e_t",
                              name=f"{lname}_et{g}")
                etv = e_t[:].rearrange("p (k h) -> p k h", h=H)
                erv = (er_t[:, g * H:(g + 1) * H]
                       .unsqueeze(1).to_broadcast((128, DEG, H)))
                nc.vector.tensor_tensor(out=etv, in0=el_e, in1=erv,
                                        op=OPT.add)
                e2 = ep.tile([128, DEG * H], f32, tag="e2",
                             name=f"{lname}_e2{g}")
                nc.vector.scalar_tensor_tensor(
                    out=e2[:], in0=e_t[:], scalar=NEG_SLOPE, in1=e_t[:],
                    op0=OPT.mult, op1=OPT.max)
                ex16 = ep.tile([128, DEG * H], f16, tag="ex16",
                               name=f"{lname}_ex16{g}")
                nc.scalar.activation(out=ex16[:], in_=e2[:], func=AF.Exp)
                den = ep.tile([128, H], f32, tag="den",
                              name=f"{lname}_den{g}")
                nc.vector.tensor_reduce(
                    out=den[:],
                    in_=ex16[:].rearrange("p (k h) -> p h k", h=H),
                    axis=AX.X, op=OPT.add)
                inv = ep.tile([128, H], f32, tag="inv",
                              name=f"{lname}_inv{g}")
                nc.vector.reciprocal(inv[:], den[:])
                # alpha-weighted sum: multiply + level-1 add on gpsimd,
                # remaining 8 slots accumulated on PE via identity matmuls
                f_all = ep.tile([128, DEG * HD], f16, tag=f"fa{HD}",
                                name=f"{lname}_fa{g}")
                if H == 1:
                    exv = (ex16[:].rearrange("p (k h) -> p k h", h=1)
                           .to_broadcast((128, DEG, HD)))
                    nc.gpsimd.tensor_tensor(
                        out=f_all[:].rearrange("p (k d) -> p k d", k=DEG),
                        in0=feat_e, in1=exv, op=OPT.mult)
                else:
                    featv = feat_e.rearrange("p k (h d) -> p k h d", h=H)
                    exv = (ex16[:].rearrange("p (k h) -> p k h", h=H)
                           .unsqueeze(3).to_broadcast((128, DEG, H, D)))
                    nc.gpsimd.tensor_tensor(
                        out=f_all[:].rearrange("p (k h d) -> p k h d",
                                               k=DEG, h=H),
                        in0=featv, in1=exv, op=OPT.mult)
                u8t = ep.tile([128, 8 * HD], f16, tag=f"u{HD}",
                              name=f"{lname}_u{g}")
                nc.gpsimd.tensor_tensor(
                    out=u8t[:], in0=f_all[:, 0:8 * HD],
                    in1=f_all[:, 8 * HD:16 * HD], op=OPT.add)
                ups = pp.tile([128, HD], f32, tag="ups",
                              name=f"{lname}_ups{g}")
                for k in range(8):
                    nc.tensor.matmul(
                        ups[:], lhsT=ident[:],
                        rhs=u8t[:, k * HD:(k + 1) * HD],
                        start=(k == 0), stop=(k == 7))
                ht = ep.tile([128, HD], f32, tag="ht",
                             name=f"{lname}_ht{g}")
                if H == 1:
                    nc.vector.scalar_tensor_tensor(
                        out=ht[:], in0=ups[:, 0:HD], scalar=inv[:, 0:1],
                        in1=b_t[:, 0:HD], op0=OPT.mult, op1=OPT.add)
                else:
                    t1 = ep.tile([128, HD], f32, tag="t1",
                                 name=f"{lname}_t1{g}")
                    invv = inv[:].unsqueeze(2).to_broadcast((128, H, D))
                    nc.vector.tensor_tensor(
                        out=t1[:].rearrange("p (h d) -> p h d", h=H),
                        in0=ups[:, 0:HD].rearrange("p (h d) -> p h d",
                                                   h=H),
                        in1=invv, op=OPT.mult)
                    nc.vector.tensor_tensor(
                        out=ht[:], in0=t1[:], in1=b_t[:, 0:HD], op=OPT.add)
                if mode == "relu":
                    hrelu = ep.tile([128, HD], f16, tag="hr",
                                    name=f"{lname}_hr{g}")
                    nc.vector.tensor_scalar_max(
                        out=hrelu[:], in0=ht[:], scalar1=0.0)
                    trp = pp.tile([128, 128], f16, tag="trp",
                                  name=f"{lname}_trp{g}")
                    nc.tensor.transpose(trp[:], hrelu[:], ident[:])
                    nc.scalar.activation(
                        out=hT_out[:, g * 128:(g + 1) * 128], in_=trp[:],
                        func=AF.Copy)
                else:  # logsoftmax (final layer)
                    r0, r1 = g * 128, (g + 1) * 128
                    nm_t = ep.tile([128, 1], f32, tag="nm",
                                   name=f"{lname}_nm{g}")
                    nc.vector.reduce_max(out=nm_t[:], in_=ht[:],
                                         axis=AX.X, negate=True)
                    exf = ep.tile([128, HD], f32, tag="exf",
                                  name=f"{lname}_exf{g}")
                    s_t = ep.tile([128, 1], f32, tag="s_t",
                                  name=f"{lname}_s{g}")
                    nc.scalar.activation(out=exf[:], in_=ht[:],
                                         func=AF.Exp, bias=nm_t[:],
                                         accum_out=s_t[:])
                    ls = ep.tile([128, 1], f32, tag="ls",
                                 name=f"{lname}_ls{g}")
                    nc.scalar.activation(out=ls[:], in_=s_t[:], func=AF.Ln)
                    o_t = ep.tile([128, HD], f32, tag="o_t",
                                  name=f"{lname}_o{g}")
                    nc.vector.scalar_tensor_tensor(
                        out=o_t[:], in0=ht[:], scalar=nm_t[:],
                        in1=ls[:].to_broadcast((128, HD)),
                        op0=OPT.add, op1=OPT.subtract)
                    nc.sync.dma_start(out=out_d[r0:r1, :], in_=o_t[:])

            def edge_loop(lname, gf_ap, blk, sub, HD, H, b_t, er_t, mode,
                          hT_out, next_feat=None, next_ag=None):
                bigtag = "big12" if blk == BLK12 else "big3"
                bigw = DEG * 2 * (SUB1 if blk == BLK12 else SUB3)
                bigs = {}
                for gi in range(G + DEFER):
                    if gi < G:
                        bigs[gi] = emit_gathers(lname, gi, gf_ap, blk, sub,
                                                bigtag, bigw)
                    g = gi - DEFER
                    if g >= 0:
                        edge_math(lname, g, bigs.pop(g), sub, HD, H, b_t,
                                  er_t, mode, hT_out)
                        if next_feat is not None:
                            next_feat(g)
                        if next_ag is not None:
                            for c in range(4):
                                if g == GCH[c + 1] - 1:
                                    next_ag(c)

            # ---- layer 1 feat + AG1 (chunks interleaved with feat) ----
            for g in range(G):
                feat_group("L1", g, [x0a, x0b], [w1a, w1b], HH + 2 * HEAD,
                           SUB1, er1, HEAD, gs_t[1])
                for c in range(4):
                    if g == GCH[c + 1] - 1:
                        ag_chunk(1, c, gf1_d)
            # ---- layer 1 edge (+ layer 2 feat + AG2 interleaved) ----
            edge_loop("L1", gf1_d, BLK12, SUB1, HH, HEAD, b1_t, er1,
                      "relu", hT1,
                      next_feat=lambda g: feat_group(
                          "L2", g, [hT1], [wh_sb], HH + 2, SUB2, er2, 1,
                          gs_t[2]),
                      next_ag=lambda c: ag_chunk(2, c, gf2_d))
            # ---- layer 2 edge (+ layer 3 feat + AG3 interleaved) ----
            edge_loop("L2", gf2_d, BLK12, SUB2, HH, 1, bh_t, er2,
                      "relu", hT2,
                      next_feat=lambda g: feat_group(
                          "L3", g, [hT2], [w2_sb], OUT + 2, SUB3, er3, 1,
                          gs_t[3]),
                      next_ag=lambda c: ag_chunk(3, c, gf3_d))
            # ---- layer 3 edge ----
            edge_loop("L3", gf3_d, BLK3, SUB3, OUT, 1, b2_t, er3,
                      "logsoftmax", None)

    nc.compile()
    return nc


# ========================================================================
# host side
# ========================================================================
def _get_program(ncores):
    if ncores not in _PROGRAM_CACHE:
        _PROGRAM_CACHE[ncores] = _build_program(ncores)
    return _PROGRAM_CACHE[ncores]


def _numpy_fallback(feats, src, dst, W1, al1, ar1, b1, Wh, alh, arh, bh,
                    W2, al2, ar2, b2):
    n = feats.shape[0]

    def gat(x, W, al, ar, b):
        Hh, Dd = al.shape
        feat = (x @ W).reshape(n, Hh, Dd)
        el = (feat * al).sum(-1)
        er = (feat * ar).sum(-1)
        e = el[src] + er[dst]
        e = np.where(e > 0, e, NEG_SLOPE * e).astype(np.float32)
        emax = np.full((n, Hh), -np.inf, np.float32)
        np.maximum.at(emax, dst, e)
        ex = np.exp(e - emax[dst])
        den = np.zeros((n, Hh), np.float32)
        np.add.at(den, dst, ex)
        alpha = ex / den[dst]
        out = np.zeros((n, Hh, Dd), np.float32)
        np.add.at(out, dst, feat[src] * alpha[..., None])
        return out + b.reshape(1, Hh, Dd)

    h = np.maximum(gat(feats, W1, al1, ar1, b1).reshape(n, HH), 0.0)
    h = np.maximum(gat(h, Wh, alh, arh, bh).mean(1), 0.0)
    h = gat(h, W2, al2, ar2, b2).mean(1)
    m = h.max(1, keepdims=True)
    ls = np.log(np.exp(h - m).sum(1, keepdims=True))
    return (h - m - ls).astype(np.float32)


def _pair_rows(src):
    """Global pair-row id + parity for each edge source, under the chunked
    AllGather table layout."""
    r = src // NV
    i = src % NV
    j = i // 2
    q = (i % 2).astype(np.uint8)
    pch = np.asarray(PCH[:4])
    pc = np.asarray(PC)
    gb = np.asarray(GB[:4])
    c = np.searchsorted(np.asarray(PCH[1:]), j, side="right")
    prow = gb[c] + r * pc[c] + (j - pch[c])
    return prow.astype(np.int16), q


def _prep_core_inputs(x0t, prow, q, r, common):
    e = prow[r * NV * DEG:(r + 1) * NV * DEG]
    eq = q[r * NV * DEG:(r + 1) * NV * DEG]
    epad = np.zeros(NS_PAD * DEG, np.int16)
    epad[:NV * DEG] = e
    eqpad = np.zeros(NS_PAD * DEG, np.uint8)
    eqpad[:NV * DEG] = eq
    ev = epad.reshape(G, 128, DEG)               # [g, p, k]
    evq = eqpad.reshape(G, 128, DEG)
    idx = np.zeros((128, G * 128), np.int16)
    sel = np.zeros((128, G * DEG), np.uint8)
    for g in range(G):
        for h in range(2):
            lst = ev[g, :, 8 * h:8 * h + 8].T.reshape(-1)  # i = j*128 + p
            a = lst.reshape(64, 16).T            # [16, 64]
            idx[:, g * 128 + h * 64:g * 128 + (h + 1) * 64] = np.tile(
                a, (8, 1))
        sel[:, g * DEG:(g + 1) * DEG] = evq[g]
    return dict(x0t=x0t, idx=idx, sel=sel, **common)


def kernel(**inputs) -> np.ndarray:
    global LAST_RESULTS
    feats = np.ascontiguousarray(np.asarray(inputs["features"],
                                            dtype=np.float32))
    src = np.asarray(inputs["src"]).astype(np.int64).ravel()
    dst = np.asarray(inputs["dst"]).astype(np.int64).ravel()
    W1 = np.asarray(inputs["W1"], dtype=np.float32)
    al1 = np.asarray(inputs["al1"], dtype=np.float32)
    ar1 = np.asarray(inputs["ar1"], dtype=np.float32)
    b1 = np.asarray(inputs["b1"], dtype=np.float32)
    Wh = np.asarray(inputs["Wh"], dtype=np.float32)
    alh = np.asarray(inputs["alh"], dtype=np.float32)
    arh = np.asarray(inputs["arh"], dtype=np.float32)
    bh = np.asarray(inputs["bh"], dtype=np.float32)
    W2 = np.asarray(inputs["W2"], dtype=np.float32)
    al2 = np.asarray(inputs["al2"], dtype=np.float32)
    ar2 = np.asarray(inputs["ar2"], dtype=np.float32)
    b2 = np.asarray(inputs["b2"], dtype=np.float32)

    n = feats.shape[0]
    expected_dst = np.repeat(np.arange(N, dtype=np.int64), DEG)
    if (n != N or src.shape[0] != N * DEG
            or not np.array_equal(dst, expected_dst)
            or src.min() < 0 or src.max() >= N):
        return _numpy_fallback(feats, src, dst, W1, al1, ar1, b1,
                               Wh, alh, arh, bh, W2, al2, ar2, b2)

    from concourse.bass_utils import run_bass_kernel_spmd

    nc = _get_program(NCORES)
    prow, q = _pair_rows(src)

    def bcast(a, w):
        return np.ascontiguousarray(
            np.broadcast_to(a.reshape(1, w), (128, w)).astype(np.float32))

    def ext16(W, al, ar):
        Hh, Dd = al.shape
        Wr = W.reshape(W.shape[0], Hh, Dd)
        wal = np.einsum("khd,hd->kh", Wr, al)
        war = np.einsum("khd,hd->kh", Wr, ar)
        return np.ascontiguousarray(
            np.concatenate([W, wal, war], axis=1).astype(np.float16))

    common = dict(
        w1=ext16(W1, al1, ar1), wh=ext16(Wh, alh, arh),
        w2=ext16(W2, al2, ar2),
        b1=bcast(b1, HH), bh=bcast(bh, HH), b2=bcast(b2, OUT),
    )
    in_maps = []
    for r in range(NCORES):
        x0t = np.zeros((IN, NS_PAD), np.float16)
        x0t[:, :NV] = feats[r * NV:(r + 1) * NV].T.astype(np.float16)
        in_maps.append(_prep_core_inputs(x0t, prow, q, r, common))

    trace = bool(int(os.environ.get("GAT_TRACE", "0")))
    LAST_RESULTS = run_bass_kernel_spmd(
        nc, in_maps, list(range(NCORES)), trace=trace)
    outs = [LAST_RESULTS.results[r]["out"][:NV] for r in range(NCORES)]
    return np.ascontiguousarray(np.concatenate(outs, axis=0),
                                dtype=np.float32)


# revision 9
# speedup vs baseline: 5.2457x; 5.2457x over previous
"""Trainium2 Bass kernel: 3-layer GAT (nn_GAT_62182536511748).

Strategy (8 NeuronCores, SPMD, fp16 pair-block gather, v3):
  - Nodes sharded contiguously across cores (6250 valid/core, padded to
    6272 = 49*128). dst == repeat(arange(N), 16): 16 in-edges per node.
  - Per layer each core computes feat = x_shard @ Wext (fp16 PE, PSUM
    fp32) where Wext = [W | W@al | W@ar] also yields el/er. Rows
    [feat|el] are packed into fp16 PAIR blocks and AllGather'd in 4 row
    chunks; the next layer's AllGather chunks are triggered from inside
    the previous layer's gather stream so the CC transfers overlap the
    gather drain.
  - Edge phase: per 128-node group (2048 edges), two 1024-index
    dma_gather instructions (int16 pair indices = src//2, rotating over
    the 4 SWDGE queues) fetch one pair block per edge. The gather drain
    (~410 descriptors/us, descriptor-count-bound) is the wall; the rest
    is spread across engines to hide underneath it:
      DVE: pair-select (copy_predicated), attention logits, softmax
           denominator, normalize + relu.
      Scalar: exp (fp16 out), PSUM->SBUF copies.
      GpSimd: alpha-weighted multiply + first reduction level.
      PE: remaining slot reduction as identity-matmul PSUM accumulation,
          feat matmuls, output transpose into the SBUF-resident hT tile
          that feeds the next layer (no DRAM round trip).
  - Edge math for group g is emitted two groups behind its gathers so
    every engine stream has slack; softmax skips max-subtraction
    (logits are O(1)).
"""

import os
import numpy as np

# ---- fixed problem dims -------------------------------------------------
N = 50000
DEG = 16
IN = 256
HID = 32
HEAD = 4
OUT = 40
HH = HID * HEAD  # 128
NEG_SLOPE = 0.2
NCORES = 8
NV = N // NCORES          # 6250 valid nodes per core
G = 49                    # groups of 128 rows
NS_PAD = G * 128          # 6272
NSB = NS_PAD // 2         # 3136 local pair rows
NBLK = NCORES * NSB       # 25088 global pair rows

# AllGather row chunks (in groups)
GCH = [0, 13, 25, 37, 49]
PCH = [g * 64 for g in GCH]               # local pair-row bounds
PC = [PCH[i + 1] - PCH[i] for i in range(4)]
GB = [NCORES * p for p in PCH]            # global pair-row bases

SUB1, SUB2, SUB3 = HH + HEAD, HH + 1, OUT + 1   # 132, 129, 41
BLK12, BLK3 = 384, 128                    # table pitch (f16 elems)
NIDX = 1024
DEFER = 2

_PROGRAM_CACHE = {}
LAST_RESULTS = None


def _dma_gather_raw(nc, mybir, out_ap, in_ap, idxs_ap, num_idxs, elem_size,
                    elem_step, queue_num=0):
    """dma_gather minus the over-strict elem%256B assert (stride must still
    be a 256B multiple; verified on HW with 528B/516B/164B elems)."""
    eng = nc.gpsimd
    stride_bytes = elem_step * mybir.dt.size(in_ap.dtype)
    assert stride_bytes % 256 == 0 and stride_bytes // 256 < 256
    _in_ap = eng.lower_ap_dma(in_ap, for_custom_bir_dma=True)
    _idxs_ap = eng.lower_ap(idxs_ap)
    _out_ap = eng.lower_ap(out_ap)
    return eng.add_instruction(
        mybir.InstDMAGatherAnt(
            name=nc.get_next_instruction_name(),
            ins=[*_in_ap, _idxs_ap,
                 eng.lower_val_access(eng.to_reg(num_idxs))],
            outs=[_out_ap],
            transpose=False, num_idxs=num_idxs, elem_size=elem_size,
            stride_bytes_256=stride_bytes // 256, gen_mode=0,
            single_packet=True, queue_num=queue_num,
            sbuf_tokens_per_rank=0, sbuf_free_dim_per_rank=0,
            sbuf_free_dim_pad_per_rank=0, sbuf_byte_offset=0,
        ))


# ========================================================================
# device program
# ========================================================================
def _build_program(ncores: int):
    from concourse import bass, mybir, tile, bacc
    from concourse.masks import make_identity
    from concourse.library_config import mlp

    f32 = mybir.dt.float32
    f16 = mybir.dt.float16
    i16 = mybir.dt.int16
    u8 = mybir.dt.uint8
    AX = mybir.AxisListType
    OPT = mybir.AluOpType
    AF = mybir.ActivationFunctionType

    nc = bacc.Bacc(
        "TRN2", target_bir_lowering=False, debug=False,
        enable_asserts=False, num_devices=ncores, num_swdge_queues=4)

    # ---- kernel I/O ----
    x0t_d = nc.dram_tensor("x0t", [IN, NS_PAD], f16, kind="ExternalInput").ap()
    idx_d = nc.dram_tensor("idx", [128, G * 128], i16,
                           kind="ExternalInput").ap()
    sel_d = nc.dram_tensor("sel", [128, G * DEG], u8,
                           kind="ExternalInput").ap()
    w1_d = nc.dram_tensor("w1", [IN, HH + 2 * HEAD], f16,
                          kind="ExternalInput").ap()
    wh_d = nc.dram_tensor("wh", [HH, HH + 2], f16, kind="ExternalInput").ap()
    w2_d = nc.dram_tensor("w2", [HH, OUT + 2], f16,
                          kind="ExternalInput").ap()
    b1_d = nc.dram_tensor("b1", [128, HH], f32, kind="ExternalInput").ap()
    bh_d = nc.dram_tensor("bh", [128, HH], f32, kind="ExternalInput").ap()
    b2_d = nc.dram_tensor("b2", [128, OUT], f32, kind="ExternalInput").ap()
    out_d = nc.dram_tensor("out", [NS_PAD, OUT], f32,
                           kind="ExternalOutput").ap()

    shared = "Shared" if ncores > 4 else "Local"
    gs_t = {}
    for l, blk in ((1, BLK12), (2, BLK12), (3, BLK3)):
        gs_t[l] = [nc.dram_tensor(f"gs{l}_{c}", [PC[c], blk], f16).ap()
                   for c in range(4)]
    gf1_d = nc.dram_tensor("gf1", [NBLK, BLK12], f16, addr_space=shared).ap()
    gf2_d = nc.dram_tensor("gf2", [NBLK, BLK12], f16, addr_space=shared).ap()
    gf3_d = nc.dram_tensor("gf3", [NBLK, BLK3], f16, addr_space=shared).ap()

    rgroups = [list(range(ncores))]

    with tile.TileContext(nc) as tc:
        with (
            tc.tile_pool(name="const", bufs=1) as cp,
            tc.tile_pool(name="feat", bufs=3) as fp,
            tc.tile_pool(name="edge", bufs=3) as ep,
            tc.tile_pool(name="psum", bufs=2, space="PSUM") as pp,
        ):
            nc.gpsimd.load_library(mlp)
            ident = cp.tile([128, 128], f16)
            make_identity(nc, ident[:])
            idx_sb = cp.tile([128, G * 128], i16)
            nc.sync.dma_start(out=idx_sb[:], in_=idx_d[:, :])
            sel_sb = cp.tile([128, G * DEG], u8)
            nc.sync.dma_start(out=sel_sb[:], in_=sel_d[:, :])
            x0a = cp.tile([128, NS_PAD], f16)
            x0b = cp.tile([128, NS_PAD], f16)
            nc.sync.dma_start(out=x0a[:], in_=x0t_d[0:128, :])
            nc.sync.dma_start(out=x0b[:], in_=x0t_d[128:256, :])
            w1a = cp.tile([128, HH + 2 * HEAD], f16)
            w1b = cp.tile([128, HH + 2 * HEAD], f16)
            nc.sync.dma_start(out=w1a[:], in_=w1_d[0:128, :])
            nc.sync.dma_start(out=w1b[:], in_=w1_d[128:256, :])
            wh_sb = cp.tile([128, HH + 2], f16)
            nc.sync.dma_start(out=wh_sb[:], in_=wh_d[:, :])
            w2_sb = cp.tile([128, OUT + 2], f16)
            nc.sync.dma_start(out=w2_sb[:], in_=w2_d[:, :])
            b1_t = cp.tile([128, HH], f32)
            nc.sync.dma_start(out=b1_t[:], in_=b1_d[:, :])
            bh_t = cp.tile([128, HH], f32)
            nc.sync.dma_start(out=bh_t[:], in_=bh_d[:, :])
            b2_t = cp.tile([128, OUT], f32)
            nc.sync.dma_start(out=b2_t[:], in_=b2_d[:, :])
            er1 = cp.tile([128, G * HEAD], f32)
            er2 = cp.tile([128, G], f32)
            er3 = cp.tile([128, G], f32)
            hT1 = cp.tile([128, NS_PAD], f16)
            hT2 = cp.tile([128, NS_PAD], f16)

            def feat_group(lname, g, lhsT_tiles, w_tiles, nw, sub, er_t, H,
                           gs_list):
                s = slice(g * 128, (g + 1) * 128)
                fps = pp.tile([128, HH + 2 * HEAD], f32, tag="fps",
                              name=f"{lname}_fps{g}")
                nchunk = len(lhsT_tiles)
                for c in range(nchunk):
                    nc.tensor.matmul(
                        fps[:, 0:nw], lhsT=lhsT_tiles[c][:, s],
                        rhs=w_tiles[c][:],
                        start=(c == 0), stop=(c == nchunk - 1))
                grow = fp.tile([128, sub], f16, tag=f"grow{lname}",
                               name=f"{lname}_grow{g}")
                nc.scalar.activation(out=grow[:], in_=fps[:, 0:sub],
                                     func=AF.Copy)
                nc.scalar.activation(out=er_t[:, g * H:(g + 1) * H],
                                     in_=fps[:, sub:sub + H], func=AF.Copy)
                c = 0
                while g >= GCH[c + 1]:
                    c += 1
                p0 = g * 64 - PCH[c]
                dst = gs_list[c][p0:p0 + 64, 0:2 * sub].rearrange(
                    "b (s c) -> b s c", c=sub)
                nc.sync.dma_start(out=dst, in_=grow[:])

            def ag_chunk(l, c, gf_ap):
                nc.gpsimd.collective_compute(
                    "AllGather", OPT.bypass, replica_groups=rgroups,
                    ins=[gs_t[l][c][:, :]],
                    outs=[gf_ap[GB[c]:GB[c + 1], :]])

            def emit_gathers(lname, g, gf_ap, blk, sub, bigtag, bigw):
                ELEM = 2 * sub
                big = ep.tile([128, bigw], f16, tag=bigtag, bufs=6,
                              name=f"{lname}_big{g}")
                for h in range(2):
                    _dma_gather_raw(
                        nc, mybir,
                        big[:, h * 8 * ELEM:(h + 1) * 8 * ELEM],
                        gf_ap[:, 0:ELEM],
                        idx_sb[:, g * 128 + h * 64:g * 128 + (h + 1) * 64],
                        NIDX, ELEM, blk, queue_num=(2 * g + h) % 4)
                return big

            def edge_math(lname, g, big, sub, HD, H, b_t, er_t, mode,
                          hT_out):
                D = HD // H
                ELEM = 2 * sub
                bv = big[:, 0:DEG * ELEM].rearrange("p (k r) -> p k r",
                                                    r=ELEM)
                lo = bv[:, :, 0:sub]
                hi = bv[:, :, sub:2 * sub]
                mask = (sel_sb[:, g * DEG:(g + 1) * DEG]
                        .unsqueeze(2).to_broadcast((128, DEG, sub)))
                nc.vector.copy_predicated(out=lo, mask=mask, data=hi)
                feat_e = bv[:, :, 0:HD]
                el_e = bv[:, :, HD:HD + H]
                # e = el + er  (er broadcast along slots)
                e_t = ep.tile([128, DEG * H], f32, tag="e_t",
                              name=f"{lname}_et{g}")
                etv = e_t[:].rearrange("p (k h) -> p k h", h=H)
                erv = (er_t[:, g * H:(g + 1) * H]
                       .unsqueeze(1).to_broadcast((128, DEG, H)))
                nc.vector.tensor_tensor(out=etv, in0=el_e, in1=erv,
                                        op=OPT.add)
                e2 = ep.tile([128, DEG * H], f32, tag="e2",
                             name=f"{lname}_e2{g}")
                nc.vector.scalar_tensor_tensor(
                    out=e2[:], in0=e_t[:], scalar=NEG_SLOPE, in1=e_t[:],
                    op0=OPT.mult, op1=OPT.max)
                ex16 = ep.tile([128, DEG * H], f16, tag="ex16",
                               name=f"{lname}_ex16{g}")
                nc.scalar.activation(out=ex16[:], in_=e2[:], func=AF.Exp)
                den = ep.tile([128, H], f32, tag="den",
                              name=f"{lname}_den{g}")
                nc.vector.tensor_reduce(
                    out=den[:],
                    in_=ex16[:].rearrange("p (k h) -> p h k", h=H),
                    axis=AX.X, op=OPT.add)
                inv = ep.tile([128, H], f32, tag="inv",
                              name=f"{lname}_inv{g}")
                nc.vector.reciprocal(inv[:], den[:])
                # alpha-weighted sum: multiply + level-1 add on gpsimd,
                # remaining 8 slots accumulated on PE via identity matmuls
                f_all = ep.tile([128, DEG * HD], f16, tag=f"fa{HD}",
                                name=f"{lname}_fa{g}")
                if H == 1:
                    exv = (ex16[:].rearrange("p (k h) -> p k h", h=1)
                           .to_broadcast((128, DEG, HD)))
                    nc.vector.tensor_tensor(
                        out=f_all[:].rearrange("p (k d) -> p k d", k=DEG),
                        in0=feat_e, in1=exv, op=OPT.mult)
                else:
                    featv = feat_e.rearrange("p k (h d) -> p k h d", h=H)
                    exv = (ex16[:].rearrange("p (k h) -> p k h", h=H)
                           .unsqueeze(3).to_broadcast((128, DEG, H, D)))
                    nc.vector.tensor_tensor(
                        out=f_all[:].rearrange("p (k h d) -> p k h d",
                                               k=DEG, h=H),
                        in0=featv, in1=exv, op=OPT.mult)
                u8t = ep.tile([128, 8 * HD], f16, tag=f"u{HD}",
                              name=f"{lname}_u{g}")
                nc.vector.tensor_tensor(
                    out=u8t[:], in0=f_all[:, 0:8 * HD],
                    in1=f_all[:, 8 * HD:16 * HD], op=OPT.add)
                ups = pp.tile([128, HD], f32, tag="ups",
                              name=f"{lname}_ups{g}")
                for k in range(8):
                    nc.tensor.matmul(
                        ups[:], lhsT=ident[:],
                        rhs=u8t[:, k * HD:(k + 1) * HD],
                        start=(k == 0), stop=(k == 7))
                ht = ep.tile([128, HD], f32, tag="ht",
                             name=f"{lname}_ht{g}")
                if H == 1:
                    nc.vector.scalar_tensor_tensor(
                        out=ht[:], in0=ups[:, 0:HD], scalar=inv[:, 0:1],
                        in1=b_t[:, 0:HD], op0=OPT.mult, op1=OPT.add)
                else:
                    t1 = ep.tile([128, HD], f32, tag="t1",
                                 name=f"{lname}_t1{g}")
                    invv = inv[:].unsqueeze(2).to_broadcast((128, H, D))
                    nc.vector.tensor_tensor(
                        out=t1[:].rearrange("p (h d) -> p h d", h=H),
                        in0=ups[:, 0:HD].rearrange("p (h d) -> p h d",
                                                   h=H),
                        in1=invv, op=OPT.mult)
                    nc.vector.tensor_tensor(
                        out=ht[:], in0=t1[:], in1=b_t[:, 0:HD], op=OPT.add)
                if mode == "relu":
                    hrelu = ep.tile([128, HD], f16, tag="hr",
                                    name=f"{lname}_hr{g}")
                    nc.vector.tensor_scalar_max(
                        out=hrelu[:], in0=ht[:], scalar1=0.0)
                    trp = pp.tile([128, 128], f16, tag="trp",
                                  name=f"{lname}_trp{g}")
                    nc.tensor.transpose(trp[:], hrelu[:], ident[:])
                    nc.scalar.activation(
                        out=hT_out[:, g * 128:(g + 1) * 128], in_=trp[:],
                        func=AF.Copy)
                else:  # logsoftmax (final layer)
                    r0, r1 = g * 128, (g + 1) * 128
                    nm_t = ep.tile([128, 1], f32, tag="nm",
                                   name=f"{lname}_nm{g}")
                    nc.vector.reduce_max(out=nm_t[:], in_=ht[:],
                                         axis=AX.X, negate=True)
                    exf = ep.tile([128, HD], f32, tag="exf",
                                  name=f"{lname}_exf{g}")
                    s_t = ep.tile([128, 1], f32, tag="s_t",
                                  name=f"{lname}_s{g}")
                    nc.scalar.activation(out=exf[:], in_=ht[:],
                                         func=AF.Exp, bias=nm_t[:],
                                         accum_out=s_t[:])
                    ls = ep.tile([128, 1], f32, tag="ls",
                                 name=f"{lname}_ls{g}")
                    nc.scalar.activation(out=ls[:], in_=s_t[:], func=AF.Ln)
                    o_t = ep.tile([128, HD], f32, tag="o_t",
                                  name=f"{lname}_o{g}")
                    nc.vector.scalar_tensor_tensor(
                        out=o_t[:], in0=ht[:], scalar=nm_t[:],
                        in1=ls[:].to_broadcast((128, HD)),
                        op0=OPT.add, op1=OPT.subtract)
                    nc.sync.dma_start(out=out_d[r0:r1, :], in_=o_t[:])

            def edge_loop(lname, gf_ap, blk, sub, HD, H, b_t, er_t, mode,
                          hT_out, next_feat=None, next_ag=None):
                bigtag = "big12" if blk == BLK12 else "big3"
                bigw = DEG * 2 * (SUB1 if blk == BLK12 else SUB3)
                bigs = {}
                for gi in range(G + DEFER):
                    if gi < G:
                        bigs[gi] = emit_gathers(lname, gi, gf_ap, blk, sub,
                                                bigtag, bigw)
                    g = gi - DEFER
                    if g >= 0:
                        edge_math(lname, g, bigs.pop(g), sub, HD, H, b_t,
                                  er_t, mode, hT_out)
                        if next_feat is not None:
                            next_feat(g)
                        if next_ag is not None:
                            for c in range(4):
                                if g == GCH[c + 1] - 1:
                                    next_ag(c)

            # ---- layer 1 feat + AG1 (chunks interleaved with feat) ----
            for g in range(G):
                feat_group("L1", g, [x0a, x0b], [w1a, w1b], HH + 2 * HEAD,
                           SUB1, er1, HEAD, gs_t[1])
                for c in range(4):
                    if g == GCH[c + 1] - 1:
                        ag_chunk(1, c, gf1_d)
            # ---- layer 1 edge (+ layer 2 feat + AG2 interleaved) ----
            edge_loop("L1", gf1_d, BLK12, SUB1, HH, HEAD, b1_t, er1,
                      "relu", hT1,
                      next_feat=lambda g: feat_group(
                          "L2", g, [hT1], [wh_sb], HH + 2, SUB2, er2, 1,
                          gs_t[2]),
                      next_ag=lambda c: ag_chunk(2, c, gf2_d))
            # ---- layer 2 edge (+ layer 3 feat + AG3 interleaved) ----
            edge_loop("L2", gf2_d, BLK12, SUB2, HH, 1, bh_t, er2,
                      "relu", hT2,
                      next_feat=lambda g: feat_group(
                          "L3", g, [hT2], [w2_sb], OUT + 2, SUB3, er3, 1,
                          gs_t[3]),
                      next_ag=lambda c: ag_chunk(3, c, gf3_d))
            # ---- layer 3 edge ----
            edge_loop("L3", gf3_d, BLK3, SUB3, OUT, 1, b2_t, er3,
                      "logsoftmax", None)

    nc.compile()
    return nc


# ========================================================================
# host side
# ========================================================================
def _get_program(ncores):
    if ncores not in _PROGRAM_CACHE:
        _PROGRAM_CACHE[ncores] = _build_program(ncores)
    return _PROGRAM_CACHE[ncores]


def _numpy_fallback(feats, src, dst, W1, al1, ar1, b1, Wh, alh, arh, bh,
                    W2, al2, ar2, b2):
    n = feats.shape[0]

    def gat(x, W, al, ar, b):
        Hh, Dd = al.shape
        feat = (x @ W).reshape(n, Hh, Dd)
        el = (feat * al).sum(-1)
        er = (feat * ar).sum(-1)
        e = el[src] + er[dst]
        e = np.where(e > 0, e, NEG_SLOPE * e).astype(np.float32)
        emax = np.full((n, Hh), -np.inf, np.float32)
        np.maximum.at(emax, dst, e)
        ex = np.exp(e - emax[dst])
        den = np.zeros((n, Hh), np.float32)
        np.add.at(den, dst, ex)
        alpha = ex / den[dst]
        out = np.zeros((n, Hh, Dd), np.float32)
        np.add.at(out, dst, feat[src] * alpha[..., None])
        return out + b.reshape(1, Hh, Dd)

    h = np.maximum(gat(feats, W1, al1, ar1, b1).reshape(n, HH), 0.0)
    h = np.maximum(gat(h, Wh, alh, arh, bh).mean(1), 0.0)
    h = gat(h, W2, al2, ar2, b2).mean(1)
    m = h.max(1, keepdims=True)
    ls = np.log(np.exp(h - m).sum(1, keepdims=True))
    return (h - m - ls).astype(np.float32)


def _pair_rows(src):
    """Global pair-row id + parity for each edge source, under the chunked
    AllGather table layout."""
    r = src // NV
    i = src % NV
    j = i // 2
    q = (i % 2).astype(np.uint8)
    pch = np.asarray(PCH[:4])
    pc = np.asarray(PC)
    gb = np.asarray(GB[:4])
    c = np.searchsorted(np.asarray(PCH[1:]), j, side="right")
    prow = gb[c] + r * pc[c] + (j - pch[c])
    return prow.astype(np.int16), q


def _prep_core_inputs(x0t, prow, q, r, common):
    e = prow[r * NV * DEG:(r + 1) * NV * DEG]
    eq = q[r * NV * DEG:(r + 1) * NV * DEG]
    epad = np.zeros(NS_PAD * DEG, np.int16)
    epad[:NV * DEG] = e
    eqpad = np.zeros(NS_PAD * DEG, np.uint8)
    eqpad[:NV * DEG] = eq
    ev = epad.reshape(G, 128, DEG)               # [g, p, k]
    evq = eqpad.reshape(G, 128, DEG)
    idx = np.zeros((128, G * 128), np.int16)
    sel = np.zeros((128, G * DEG), np.uint8)
    for g in range(G):
        for h in range(2):
            lst = ev[g, :, 8 * h:8 * h + 8].T.reshape(-1)  # i = j*128 + p
            a = lst.reshape(64, 16).T            # [16, 64]
            idx[:, g * 128 + h * 64:g * 128 + (h + 1) * 64] = np.tile(
                a, (8, 1))
        sel[:, g * DEG:(g + 1) * DEG] = evq[g]
    return dict(x0t=x0t, idx=idx, sel=sel, **common)


def kernel(**inputs) -> np.ndarray:
    global LAST_RESULTS
    feats = np.ascontiguousarray(np.asarray(inputs["features"],
                                            dtype=np.float32))
    src = np.asarray(inputs["src"]).astype(np.int64).ravel()
    dst = np.asarray(inputs["dst"]).astype(np.int64).ravel()
    W1 = np.asarray(inputs["W1"], dtype=np.float32)
    al1 = np.asarray(inputs["al1"], dtype=np.float32)
    ar1 = np.asarray(inputs["ar1"], dtype=np.float32)
    b1 = np.asarray(inputs["b1"], dtype=np.float32)
    Wh = np.asarray(inputs["Wh"], dtype=np.float32)
    alh = np.asarray(inputs["alh"], dtype=np.float32)
    arh = np.asarray(inputs["arh"], dtype=np.float32)
    bh = np.asarray(inputs["bh"], dtype=np.float32)
    W2 = np.asarray(inputs["W2"], dtype=np.float32)
    al2 = np.asarray(inputs["al2"], dtype=np.float32)
    ar2 = np.asarray(inputs["ar2"], dtype=np.float32)
    b2 = np.asarray(inputs["b2"], dtype=np.float32)

    n = feats.shape[0]
    expected_dst = np.repeat(np.arange(N, dtype=np.int64), DEG)
    if (n != N or src.shape[0] != N * DEG
            or not np.array_equal(dst, expected_dst)
            or src.min() < 0 or src.max() >= N):
        return _numpy_fallback(feats, src, dst, W1, al1, ar1, b1,
                               Wh, alh, arh, bh, W2, al2, ar2, b2)

    from concourse.bass_utils import run_bass_kernel_spmd

    nc = _get_program(NCORES)
    prow, q = _pair_rows(src)

    def bcast(a, w):
        return np.ascontiguousarray(
            np.broadcast_to(a.reshape(1, w), (128, w)).astype(np.float32))

    def ext16(W, al, ar):
        Hh, Dd = al.shape
        Wr = W.reshape(W.shape[0], Hh, Dd)
        wal = np.einsum("khd,hd->kh", Wr, al)
        war = np.einsum("khd,hd->kh", Wr, ar)
        return np.ascontiguousarray(
            np.concatenate([W, wal, war], axis=1).astype(np.float16))

    common = dict(
        w1=ext16(W1, al1, ar1), wh=ext16(Wh, alh, arh),
        w2=ext16(W2, al2, ar2),
        b1=bcast(b1, HH), bh=bcast(bh, HH), b2=bcast(b2, OUT),
    )
    in_maps = []
    for r in range(NCORES):
        x0t = np.zeros((IN, NS_PAD), np.float16)
        x0t[:, :NV] = feats[r * NV:(r + 1) * NV].T.astype(np.float16)
        in_maps.append(_prep_core_inputs(x0t, prow, q, r, common))

    trace = bool(int(os.environ.get("GAT_TRACE", "0")))
    LAST_RESULTS = run_bass_kernel_spmd(
        nc, in_maps, list(range(NCORES)), trace=trace)
    outs = [LAST_RESULTS.results[r]["out"][:NV] for r in range(NCORES)]
    return np.ascontiguousarray(np.concatenate(outs, axis=0),
                                dtype=np.float32)


# revision 11
# speedup vs baseline: 6.8045x; 1.2972x over previous
"""Trainium2 Bass kernel: 3-layer GAT (nn_GAT_62182536511748).

Strategy (8 NeuronCores, SPMD, fp16 pair-block gather, v3):
  - Nodes sharded contiguously across cores (6250 valid/core, padded to
    6272 = 49*128). dst == repeat(arange(N), 16): 16 in-edges per node.
  - Per layer each core computes feat = x_shard @ Wext (fp16 PE, PSUM
    fp32) where Wext = [W | W@al | W@ar] also yields el/er. Rows
    [feat|el] are packed into fp16 PAIR blocks and AllGather'd in 4 row
    chunks; the next layer's AllGather chunks are triggered from inside
    the previous layer's gather stream so the CC transfers overlap the
    gather drain.
  - Edge phase: per 128-node group (2048 edges), two 1024-index
    dma_gather instructions (int16 pair indices = src//2, rotating over
    the 4 SWDGE queues) fetch one pair block per edge. The gather drain
    (~410 descriptors/us, descriptor-count-bound) is the wall; the rest
    is spread across engines to hide underneath it:
      DVE: pair-select (copy_predicated), attention logits, softmax
           denominator, normalize + relu.
      Scalar: exp (fp16 out), PSUM->SBUF copies.
      GpSimd: alpha-weighted multiply + first reduction level.
      PE: remaining slot reduction as identity-matmul PSUM accumulation,
          feat matmuls, output transpose into the SBUF-resident hT tile
          that feeds the next layer (no DRAM round trip).
  - Edge math for group g is emitted two groups behind its gathers so
    every engine stream has slack; softmax skips max-subtraction
    (logits are O(1)).
"""

import os
import numpy as np

# ---- fixed problem dims -------------------------------------------------
N = 50000
DEG = 16
IN = 256
HID = 32
HEAD = 4
OUT = 40
HH = HID * HEAD  # 128
NEG_SLOPE = 0.2
NCORES = 8
NV = N // NCORES          # 6250 valid nodes per core
G = 49                    # groups of 128 rows
NS_PAD = G * 128          # 6272
NSB = NS_PAD // 2         # 3136 local pair rows
NBLK = NCORES * NSB       # 25088 global pair rows

# AllGather row chunks (in groups)
GCH = [0, 13, 25, 37, 49]
PCH = [g * 64 for g in GCH]               # local pair-row bounds
PC = [PCH[i + 1] - PCH[i] for i in range(4)]
GB = [NCORES * p for p in PCH]            # global pair-row bases

SUB1, SUB2, SUB3 = HH + HEAD, HH + 1, OUT + 1   # 132, 129, 41
BLK12, BLK3 = 384, 128                    # table pitch (f16 elems)
NIDX = 1024
DEFER = 3

_PROGRAM_CACHE = {}
LAST_RESULTS = None


def _dma_gather_raw(nc, mybir, out_ap, in_ap, idxs_ap, num_idxs, elem_size,
                    elem_step, queue_num=0):
    """dma_gather minus the over-strict elem%256B assert (stride must still
    be a 256B multiple; verified on HW with 528B/516B/164B elems)."""
    eng = nc.gpsimd
    stride_bytes = elem_step * mybir.dt.size(in_ap.dtype)
    assert stride_bytes % 256 == 0 and stride_bytes // 256 < 256
    _in_ap = eng.lower_ap_dma(in_ap, for_custom_bir_dma=True)
    _idxs_ap = eng.lower_ap(idxs_ap)
    _out_ap = eng.lower_ap(out_ap)
    return eng.add_instruction(
        mybir.InstDMAGatherAnt(
            name=nc.get_next_instruction_name(),
            ins=[*_in_ap, _idxs_ap,
                 eng.lower_val_access(eng.to_reg(num_idxs))],
            outs=[_out_ap],
            transpose=False, num_idxs=num_idxs, elem_size=elem_size,
            stride_bytes_256=stride_bytes // 256, gen_mode=0,
            single_packet=True, queue_num=queue_num,
            sbuf_tokens_per_rank=0, sbuf_free_dim_per_rank=0,
            sbuf_free_dim_pad_per_rank=0, sbuf_byte_offset=0,
        ))


# ========================================================================
# device program
# ========================================================================
def _build_program(ncores: int):
    from concourse import bass, mybir, tile, bacc
    from concourse.masks import make_identity
    from concourse.library_config import mlp

    f32 = mybir.dt.float32
    f16 = mybir.dt.float16
    i16 = mybir.dt.int16
    u8 = mybir.dt.uint8
    AX = mybir.AxisListType
    OPT = mybir.AluOpType
    AF = mybir.ActivationFunctionType

    nc = bacc.Bacc(
        "TRN2", target_bir_lowering=False, debug=False,
        enable_asserts=False, num_devices=ncores, num_swdge_queues=4)

    # ---- kernel I/O ----
    x0t_d = nc.dram_tensor("x0t", [IN, NS_PAD], f16, kind="ExternalInput").ap()
    idx_d = nc.dram_tensor("idx", [128, G * 128], i16,
                           kind="ExternalInput").ap()
    sel_d = nc.dram_tensor("sel", [128, G * DEG], u8,
                           kind="ExternalInput").ap()
    w1_d = nc.dram_tensor("w1", [IN, HH + 2 * HEAD], f16,
                          kind="ExternalInput").ap()
    wh_d = nc.dram_tensor("wh", [HH, HH + 2], f16, kind="ExternalInput").ap()
    w2_d = nc.dram_tensor("w2", [HH, OUT + 2], f16,
                          kind="ExternalInput").ap()
    b1_d = nc.dram_tensor("b1", [128, HH], f32, kind="ExternalInput").ap()
    bh_d = nc.dram_tensor("bh", [128, HH], f32, kind="ExternalInput").ap()
    b2_d = nc.dram_tensor("b2", [128, OUT], f32, kind="ExternalInput").ap()
    out_d = nc.dram_tensor("out", [NS_PAD, OUT], f32,
                           kind="ExternalOutput").ap()

    shared = "Shared" if ncores > 4 else "Local"
    gs_t = {}
    for l, blk in ((1, BLK12), (2, BLK12), (3, BLK3)):
        gs_t[l] = [nc.dram_tensor(f"gs{l}_{c}", [PC[c], blk], f16).ap()
                   for c in range(4)]
    gf1_d = nc.dram_tensor("gf1", [NBLK, BLK12], f16, addr_space=shared).ap()
    gf2_d = nc.dram_tensor("gf2", [NBLK, BLK12], f16, addr_space=shared).ap()
    gf3_d = nc.dram_tensor("gf3", [NBLK, BLK3], f16, addr_space=shared).ap()

    rgroups = [list(range(ncores))]

    with tile.TileContext(nc) as tc:
        with (
            tc.tile_pool(name="const", bufs=1) as cp,
            tc.tile_pool(name="feat", bufs=3) as fp,
            tc.tile_pool(name="edge", bufs=3) as ep,
            tc.tile_pool(name="psum", bufs=2, space="PSUM") as pp,
        ):
            nc.gpsimd.load_library(mlp)
            ident = cp.tile([128, 128], f16)
            make_identity(nc, ident[:])
            idx_sb = cp.tile([128, G * 128], i16)
            nc.sync.dma_start(out=idx_sb[:], in_=idx_d[:, :])
            sel_sb = cp.tile([128, G * DEG], u8)
            nc.sync.dma_start(out=sel_sb[:], in_=sel_d[:, :])
            x0a = cp.tile([128, NS_PAD], f16)
            x0b = cp.tile([128, NS_PAD], f16)
            nc.sync.dma_start(out=x0a[:], in_=x0t_d[0:128, :])
            nc.sync.dma_start(out=x0b[:], in_=x0t_d[128:256, :])
            w1a = cp.tile([128, HH + 2 * HEAD], f16)
            w1b = cp.tile([128, HH + 2 * HEAD], f16)
            nc.sync.dma_start(out=w1a[:], in_=w1_d[0:128, :])
            nc.sync.dma_start(out=w1b[:], in_=w1_d[128:256, :])
            wh_sb = cp.tile([128, HH + 2], f16)
            nc.sync.dma_start(out=wh_sb[:], in_=wh_d[:, :])
            w2_sb = cp.tile([128, OUT + 2], f16)
            nc.sync.dma_start(out=w2_sb[:], in_=w2_d[:, :])
            b1_t = cp.tile([128, HH], f32)
            nc.sync.dma_start(out=b1_t[:], in_=b1_d[:, :])
            bh_t = cp.tile([128, HH], f32)
            nc.sync.dma_start(out=bh_t[:], in_=bh_d[:, :])
            b2_t = cp.tile([128, OUT], f32)
            nc.sync.dma_start(out=b2_t[:], in_=b2_d[:, :])
            er1 = cp.tile([128, G * HEAD], f32)
            er2 = cp.tile([128, G], f32)
            er3 = cp.tile([128, G], f32)
            hT1 = cp.tile([128, NS_PAD], f16)
            hT2 = cp.tile([128, NS_PAD], f16)

            def feat_group(lname, g, lhsT_tiles, w_tiles, nw, sub, er_t, H,
                           gs_list):
                s = slice(g * 128, (g + 1) * 128)
                fps = pp.tile([128, HH + 2 * HEAD], f32, tag="fps",
                              name=f"{lname}_fps{g}")
                nchunk = len(lhsT_tiles)
                for c in range(nchunk):
                    nc.tensor.matmul(
                        fps[:, 0:nw], lhsT=lhsT_tiles[c][:, s],
                        rhs=w_tiles[c][:],
                        start=(c == 0), stop=(c == nchunk - 1))
                grow = fp.tile([128, sub], f16, tag=f"grow{lname}",
                               name=f"{lname}_grow{g}")
                nc.scalar.activation(out=grow[:], in_=fps[:, 0:sub],
                                     func=AF.Copy)
                nc.scalar.activation(out=er_t[:, g * H:(g + 1) * H],
                                     in_=fps[:, sub:sub + H], func=AF.Copy)
                c = 0
                while g >= GCH[c + 1]:
                    c += 1
                p0 = g * 64 - PCH[c]
                dst = gs_list[c][p0:p0 + 64, 0:2 * sub].rearrange(
                    "b (s c) -> b s c", c=sub)
                nc.sync.dma_start(out=dst, in_=grow[:])

            def ag_chunk(l, c, gf_ap):
                nc.gpsimd.collective_compute(
                    "AllGather", OPT.bypass, replica_groups=rgroups,
                    ins=[gs_t[l][c][:, :]],
                    outs=[gf_ap[GB[c]:GB[c + 1], :]])

            def emit_gathers(lname, g, gf_ap, blk, sub, bigtag, bigw):
                ELEM = 2 * sub
                big = ep.tile([128, bigw], f16, tag=bigtag, bufs=6,
                              name=f"{lname}_big{g}")
                for h in range(2):
                    _dma_gather_raw(
                        nc, mybir,
                        big[:, h * 8 * ELEM:(h + 1) * 8 * ELEM],
                        gf_ap[:, 0:ELEM],
                        idx_sb[:, g * 128 + h * 64:g * 128 + (h + 1) * 64],
                        NIDX, ELEM, blk, queue_num=(2 * g + h) % 4)
                return big

            def edge_math(lname, g, big, sub, HD, H, b_t, er_t, mode,
                          hT_out):
                D = HD // H
                ELEM = 2 * sub
                bv = big[:, 0:DEG * ELEM].rearrange("p (k r) -> p k r",
                                                    r=ELEM)
                lo = bv[:, :, 0:sub]
                hi = bv[:, :, sub:2 * sub]
                mask = (sel_sb[:, g * DEG:(g + 1) * DEG]
                        .unsqueeze(2).to_broadcast((128, DEG, sub)))
                nc.vector.copy_predicated(out=lo, mask=mask, data=hi)
                feat_e = bv[:, :, 0:HD]
                el_e = bv[:, :, HD:HD + H]
                # e = el + er  (er broadcast along slots)
                e_t = ep.tile([128, DEG * H], f32, tag="e_t",
                              name=f"{lname}_et{g}")
                etv = e_t[:].rearrange("p (k h) -> p k h", h=H)
                erv = (er_t[:, g * H:(g + 1) * H]
                       .unsqueeze(1).to_broadcast((128, DEG, H)))
                nc.vector.tensor_tensor(out=etv, in0=el_e, in1=erv,
                                        op=OPT.add)
                e2 = ep.tile([128, DEG * H], f32, tag="e2",
                             name=f"{lname}_e2{g}")
                nc.vector.scalar_tensor_tensor(
                    out=e2[:], in0=e_t[:], scalar=NEG_SLOPE, in1=e_t[:],
                    op0=OPT.mult, op1=OPT.max)
                ex16 = ep.tile([128, DEG * H], f16, tag="ex16",
                               name=f"{lname}_ex16{g}")
                nc.scalar.activation(out=ex16[:], in_=e2[:], func=AF.Exp)
                den = ep.tile([128, H], f32, tag="den",
                              name=f"{lname}_den{g}")
                nc.vector.tensor_reduce(
                    out=den[:],
                    in_=ex16[:].rearrange("p (k h) -> p h k", h=H),
                    axis=AX.X, op=OPT.add)
                inv = ep.tile([128, H], f32, tag="inv",
                              name=f"{lname}_inv{g}")
                nc.vector.reciprocal(inv[:], den[:])
                # alpha-weighted sum: multiply + level-1 add on gpsimd,
                # remaining 8 slots accumulated on PE via identity matmuls
                f_all = ep.tile([128, DEG * HD], f16, tag=f"fa{HD}",
                                name=f"{lname}_fa{g}")
                if H == 1:
                    exv = (ex16[:].rearrange("p (k h) -> p k h", h=1)
                           .to_broadcast((128, DEG, HD)))
                    nc.vector.tensor_tensor(
                        out=f_all[:].rearrange("p (k d) -> p k d", k=DEG),
                        in0=feat_e, in1=exv, op=OPT.mult)
                else:
                    featv = feat_e.rearrange("p k (h d) -> p k h d", h=H)
                    exv = (ex16[:].rearrange("p (k h) -> p k h", h=H)
                           .unsqueeze(3).to_broadcast((128, DEG, H, D)))
                    nc.vector.tensor_tensor(
                        out=f_all[:].rearrange("p (k h d) -> p k h d",
                                               k=DEG, h=H),
                        in0=featv, in1=exv, op=OPT.mult)
                u8t = ep.tile([128, 8 * HD], f16, tag=f"u{HD}",
                              name=f"{lname}_u{g}")
                nc.vector.tensor_tensor(
                    out=u8t[:], in0=f_all[:, 0:8 * HD],
                    in1=f_all[:, 8 * HD:16 * HD], op=OPT.add)
                ups = pp.tile([128, HD], f32, tag="ups",
                              name=f"{lname}_ups{g}")
                for k in range(8):
                    nc.tensor.matmul(
                        ups[:], lhsT=ident[:],
                        rhs=u8t[:, k * HD:(k + 1) * HD],
                        start=(k == 0), stop=(k == 7))
                ht = ep.tile([128, HD], f32, tag="ht",
                             name=f"{lname}_ht{g}")
                if H == 1:
                    nc.vector.scalar_tensor_tensor(
                        out=ht[:], in0=ups[:, 0:HD], scalar=inv[:, 0:1],
                        in1=b_t[:, 0:HD], op0=OPT.mult, op1=OPT.add)
                else:
                    t1 = ep.tile([128, HD], f32, tag="t1",
                                 name=f"{lname}_t1{g}")
                    invv = inv[:].unsqueeze(2).to_broadcast((128, H, D))
                    nc.vector.tensor_tensor(
                        out=t1[:].rearrange("p (h d) -> p h d", h=H),
                        in0=ups[:, 0:HD].rearrange("p (h d) -> p h d",
                                                   h=H),
                        in1=invv, op=OPT.mult)
                    nc.vector.tensor_tensor(
                        out=ht[:], in0=t1[:], in1=b_t[:, 0:HD], op=OPT.add)
                if mode == "relu":
                    hrelu = ep.tile([128, HD], f16, tag="hr",
                                    name=f"{lname}_hr{g}")
                    nc.vector.scalar_tensor_tensor(
                        out=hrelu[:], in0=ht[:], scalar=0.0, in1=ht[:],
                        op0=OPT.max, op1=OPT.max)
                    trp = pp.tile([128, 128], f16, tag="trp",
                                  name=f"{lname}_trp{g}")
                    nc.tensor.transpose(trp[:], hrelu[:], ident[:])
                    nc.scalar.activation(
                        out=hT_out[:, g * 128:(g + 1) * 128], in_=trp[:],
                        func=AF.Copy)
                else:  # logsoftmax (final layer)
                    r0, r1 = g * 128, (g + 1) * 128
                    nm_t = ep.tile([128, 1], f32, tag="nm",
                                   name=f"{lname}_nm{g}")
                    nc.vector.reduce_max(out=nm_t[:], in_=ht[:],
                                         axis=AX.X, negate=True)
                    exf = ep.tile([128, HD], f32, tag="exf",
                                  name=f"{lname}_exf{g}")
                    s_t = ep.tile([128, 1], f32, tag="s_t",
                                  name=f"{lname}_s{g}")
                    nc.scalar.activation(out=exf[:], in_=ht[:],
                                         func=AF.Exp, bias=nm_t[:],
                                         accum_out=s_t[:])
                    ls = ep.tile([128, 1], f32, tag="ls",
                                 name=f"{lname}_ls{g}")
                    nc.scalar.activation(out=ls[:], in_=s_t[:], func=AF.Ln)
                    o_t = ep.tile([128, HD], f32, tag="o_t",
                                  name=f"{lname}_o{g}")
                    nc.vector.scalar_tensor_tensor(
                        out=o_t[:], in0=ht[:], scalar=nm_t[:],
                        in1=ls[:].to_broadcast((128, HD)),
                        op0=OPT.add, op1=OPT.subtract)
                    nc.sync.dma_start(out=out_d[r0:r1, :], in_=o_t[:])

            def edge_loop(lname, gf_ap, blk, sub, HD, H, b_t, er_t, mode,
                          hT_out, next_feat=None, next_ag=None):
                bigtag = "big12" if blk == BLK12 else "big3"
                bigw = DEG * 2 * (SUB1 if blk == BLK12 else SUB3)
                bigs = {}
                for gi in range(G + DEFER):
                    if gi < G:
                        bigs[gi] = emit_gathers(lname, gi, gf_ap, blk, sub,
                                                bigtag, bigw)
                    g = gi - DEFER
                    if g >= 0:
                        edge_math(lname, g, bigs.pop(g), sub, HD, H, b_t,
                                  er_t, mode, hT_out)
                        if next_feat is not None:
                            next_feat(g)
                        if next_ag is not None:
                            for c in range(4):
                                if g == GCH[c + 1] - 1:
                                    next_ag(c)

            # ---- layer 1 feat + AG1 (chunks interleaved with feat) ----
            for g in range(G):
                feat_group("L1", g, [x0a, x0b], [w1a, w1b], HH + 2 * HEAD,
                           SUB1, er1, HEAD, gs_t[1])
                for c in range(4):
                    if g == GCH[c + 1] - 1:
                        ag_chunk(1, c, gf1_d)
            # ---- layer 1 edge (+ layer 2 feat + AG2 interleaved) ----
            edge_loop("L1", gf1_d, BLK12, SUB1, HH, HEAD, b1_t, er1,
                      "relu", hT1,
                      next_feat=lambda g: feat_group(
                          "L2", g, [hT1], [wh_sb], HH + 2, SUB2, er2, 1,
                          gs_t[2]),
                      next_ag=lambda c: ag_chunk(2, c, gf2_d))
            # ---- layer 2 edge (+ layer 3 feat + AG3 interleaved) ----
            edge_loop("L2", gf2_d, BLK12, SUB2, HH, 1, bh_t, er2,
                      "relu", hT2,
                      next_feat=lambda g: feat_group(
                          "L3", g, [hT2], [w2_sb], OUT + 2, SUB3, er3, 1,
                          gs_t[3]),
                      next_ag=lambda c: ag_chunk(3, c, gf3_d))
            # ---- layer 3 edge ----
            edge_loop("L3", gf3_d, BLK3, SUB3, OUT, 1, b2_t, er3,
                      "logsoftmax", None)

    nc.compile()
    return nc


# ========================================================================
# host side
# ========================================================================
def _get_program(ncores):
    if ncores not in _PROGRAM_CACHE:
        _PROGRAM_CACHE[ncores] = _build_program(ncores)
    return _PROGRAM_CACHE[ncores]


def _numpy_fallback(feats, src, dst, W1, al1, ar1, b1, Wh, alh, arh, bh,
                    W2, al2, ar2, b2):
    n = feats.shape[0]

    def gat(x, W, al, ar, b):
        Hh, Dd = al.shape
        feat = (x @ W).reshape(n, Hh, Dd)
        el = (feat * al).sum(-1)
        er = (feat * ar).sum(-1)
        e = el[src] + er[dst]
        e = np.where(e > 0, e, NEG_SLOPE * e).astype(np.float32)
        emax = np.full((n, Hh), -np.inf, np.float32)
        np.maximum.at(emax, dst, e)
        ex = np.exp(e - emax[dst])
        den = np.zeros((n, Hh), np.float32)
        np.add.at(den, dst, ex)
        alpha = ex / den[dst]
        out = np.zeros((n, Hh, Dd), np.float32)
        np.add.at(out, dst, feat[src] * alpha[..., None])
        return out + b.reshape(1, Hh, Dd)

    h = np.maximum(gat(feats, W1, al1, ar1, b1).reshape(n, HH), 0.0)
    h = np.maximum(gat(h, Wh, alh, arh, bh).mean(1), 0.0)
    h = gat(h, W2, al2, ar2, b2).mean(1)
    m = h.max(1, keepdims=True)
    ls = np.log(np.exp(h - m).sum(1, keepdims=True))
    return (h - m - ls).astype(np.float32)


def _pair_rows(src):
    """Global pair-row id + parity for each edge source, under the chunked
    AllGather table layout."""
    r = src // NV
    i = src % NV
    j = i // 2
    q = (i % 2).astype(np.uint8)
    pch = np.asarray(PCH[:4])
    pc = np.asarray(PC)
    gb = np.asarray(GB[:4])
    c = np.searchsorted(np.asarray(PCH[1:]), j, side="right")
    prow = gb[c] + r * pc[c] + (j - pch[c])
    return prow.astype(np.int16), q


def _prep_core_inputs(x0t, prow, q, r, common):
    e = prow[r * NV * DEG:(r + 1) * NV * DEG]
    eq = q[r * NV * DEG:(r + 1) * NV * DEG]
    epad = np.zeros(NS_PAD * DEG, np.int16)
    epad[:NV * DEG] = e
    eqpad = np.zeros(NS_PAD * DEG, np.uint8)
    eqpad[:NV * DEG] = eq
    ev = epad.reshape(G, 128, DEG)               # [g, p, k]
    evq = eqpad.reshape(G, 128, DEG)
    idx = np.zeros((128, G * 128), np.int16)
    sel = np.zeros((128, G * DEG), np.uint8)
    for g in range(G):
        for h in range(2):
            lst = ev[g, :, 8 * h:8 * h + 8].T.reshape(-1)  # i = j*128 + p
            a = lst.reshape(64, 16).T            # [16, 64]
            idx[:, g * 128 + h * 64:g * 128 + (h + 1) * 64] = np.tile(
                a, (8, 1))
        sel[:, g * DEG:(g + 1) * DEG] = evq[g]
    return dict(x0t=x0t, idx=idx, sel=sel, **common)


def kernel(**inputs) -> np.ndarray:
    global LAST_RESULTS
    feats = np.ascontiguousarray(np.asarray(inputs["features"],
                                            dtype=np.float32))
    src = np.asarray(inputs["src"]).astype(np.int64).ravel()
    dst = np.asarray(inputs["dst"]).astype(np.int64).ravel()
    W1 = np.asarray(inputs["W1"], dtype=np.float32)
    al1 = np.asarray(inputs["al1"], dtype=np.float32)
    ar1 = np.asarray(inputs["ar1"], dtype=np.float32)
    b1 = np.asarray(inputs["b1"], dtype=np.float32)
    Wh = np.asarray(inputs["Wh"], dtype=np.float32)
    alh = np.asarray(inputs["alh"], dtype=np.float32)
    arh = np.asarray(inputs["arh"], dtype=np.float32)
    bh = np.asarray(inputs["bh"], dtype=np.float32)
    W2 = np.asarray(inputs["W2"], dtype=np.float32)
    al2 = np.asarray(inputs["al2"], dtype=np.float32)
    ar2 = np.asarray(inputs["ar2"], dtype=np.float32)
    b2 = np.asarray(inputs["b2"], dtype=np.float32)

    n = feats.shape[0]
    expected_dst = np.repeat(np.arange(N, dtype=np.int64), DEG)
    if (n != N or src.shape[0] != N * DEG
            or not np.array_equal(dst, expected_dst)
            or src.min() < 0 or src.max() >= N):
        return _numpy_fallback(feats, src, dst, W1, al1, ar1, b1,
                               Wh, alh, arh, bh, W2, al2, ar2, b2)

    from concourse.bass_utils import run_bass_kernel_spmd

    nc = _get_program(NCORES)
    prow, q = _pair_rows(src)

    def bcast(a, w):
        return np.ascontiguousarray(
            np.broadcast_to(a.reshape(1, w), (128, w)).astype(np.float32))

    def ext16(W, al, ar):
        Hh, Dd = al.shape
        Wr = W.reshape(W.shape[0], Hh, Dd)
        wal = np.einsum("khd,hd->kh", Wr, al)
        war = np.einsum("khd,hd->kh", Wr, ar)
        return np.ascontiguousarray(
            np.concatenate([W, wal, war], axis=1).astype(np.float16))

    common = dict(
        w1=ext16(W1, al1, ar1), wh=ext16(Wh, alh, arh),
        w2=ext16(W2, al2, ar2),
        b1=bcast(b1, HH), bh=bcast(bh, HH), b2=bcast(b2, OUT),
    )
    in_maps = []
    for r in range(NCORES):
        x0t = np.zeros((IN, NS_PAD), np.float16)
        x0t[:, :NV] = feats[r * NV:(r + 1) * NV].T.astype(np.float16)
        in_maps.append(_prep_core_inputs(x0t, prow, q, r, common))

    trace = bool(int(os.environ.get("GAT_TRACE", "0")))
    LAST_RESULTS = run_bass_kernel_spmd(
        nc, in_maps, list(range(NCORES)), trace=trace)
    outs = [LAST_RESULTS.results[r]["out"][:NV] for r in range(NCORES)]
    return np.ascontiguousarray(np.concatenate(outs, axis=0),
                                dtype=np.float32)


# revision 12
# speedup vs baseline: 7.6956x; 1.1310x over previous
"""Trainium2 Bass kernel: 3-layer GAT (nn_GAT_62182536511748).

Strategy (8 NeuronCores, SPMD, pair-block gather, v6):
  - Nodes sharded contiguously across cores (6250 valid/core, padded to
    6272 = 49*128). dst == repeat(arange(N), 16): 16 in-edges per node.
  - Per layer each core computes feat = x_shard @ Wext (fp16 PE, PSUM
    fp32) where Wext = [W | W@al | W@ar] also yields el/er. Rows
    [feat|el] are packed into PAIR blocks (fp8 e4m3 feat + fp16 el for
    layers 1-2, fp16 for layer 3) and AllGather'd: layer 1 in one shot
    (head), layers 2-3 in 5 row chunks triggered from inside the
    previous layer's gather stream so the CC transfers overlap the
    gather drain.
  - Edge phase: per 128-node group (2048 edges), two 1024-index
    dma_gather instructions (int16 pair indices = src//2, rotating over
    the 4 SWDGE queues) fetch one pair block per edge. The gather drain
    (~440 descriptors/us, descriptor-count-bound) is the wall; the rest
    is spread across engines to hide underneath it:
      DVE: pair-select (copy_predicated), attention logits, softmax
           denominator, alpha multiply + first reduction level,
           normalize + relu.
      Scalar: exp (fp16 out), PSUM->SBUF packing copies.
      PE: remaining slot reduction as identity-matmul PSUM accumulation,
          feat matmuls, output transpose into the SBUF-resident hT tile
          that feeds the next layer (no DRAM round trip).
  - Edge math for group g is emitted three groups behind its gathers so
    every engine stream has slack; softmax skips max-subtraction
    (logits are O(1)).
"""

import os
import numpy as np

# ---- fixed problem dims -------------------------------------------------
N = 50000
DEG = 16
IN = 256
HID = 32
HEAD = 4
OUT = 40
HH = HID * HEAD  # 128
NEG_SLOPE = 0.2
NCORES = 8
NV = N // NCORES          # 6250 valid nodes per core
G = 49                    # groups of 128 rows
NS_PAD = G * 128          # 6272
NSB = NS_PAD // 2         # 3136 local pair rows
NBLK = NCORES * NSB       # 25088 global pair rows

# AllGather row chunks (in groups): L1 single-shot, L2/L3 5 chunks with a
# small last chunk to shrink the layer-boundary bubble.
CHB1 = [0, 49]
CHB23 = [0, 12, 24, 34, 44, 49]

FD = 64                    # fp8 feat payload in f16-container elems
SUBC1, SUBC2 = FD + HEAD, FD + 1   # 68, 65 container elems per half
SUB3 = OUT + 1             # 41 (fp16)
BLK12, BLK3 = 256, 128     # table pitch (f16 elems): 512B / 256B
NIDX = 1024
DEFER = 3

_PROGRAM_CACHE = {}
LAST_RESULTS = None


def _dma_gather_raw(nc, mybir, out_ap, in_ap, idxs_ap, num_idxs, elem_size,
                    elem_step, queue_num=0):
    """dma_gather minus the over-strict elem%256B assert (stride must still
    be a 256B multiple; verified on HW with 528B/272B/164B elems)."""
    eng = nc.gpsimd
    stride_bytes = elem_step * mybir.dt.size(in_ap.dtype)
    assert stride_bytes % 256 == 0 and stride_bytes // 256 < 256
    _in_ap = eng.lower_ap_dma(in_ap, for_custom_bir_dma=True)
    _idxs_ap = eng.lower_ap(idxs_ap)
    _out_ap = eng.lower_ap(out_ap)
    return eng.add_instruction(
        mybir.InstDMAGatherAnt(
            name=nc.get_next_instruction_name(),
            ins=[*_in_ap, _idxs_ap,
                 eng.lower_val_access(eng.to_reg(num_idxs))],
            outs=[_out_ap],
            transpose=False, num_idxs=num_idxs, elem_size=elem_size,
            stride_bytes_256=stride_bytes // 256, gen_mode=0,
            single_packet=True, queue_num=queue_num,
            sbuf_tokens_per_rank=0, sbuf_free_dim_per_rank=0,
            sbuf_free_dim_pad_per_rank=0, sbuf_byte_offset=0,
        ))


# ========================================================================
# device program
# ========================================================================
def _build_program(ncores: int):
    from concourse import bass, mybir, tile, bacc
    from concourse.masks import make_identity
    from concourse.library_config import mlp

    f32 = mybir.dt.float32
    f16 = mybir.dt.float16
    f8 = mybir.dt.float8e4
    i16 = mybir.dt.int16
    u8 = mybir.dt.uint8
    AX = mybir.AxisListType
    OPT = mybir.AluOpType
    AF = mybir.ActivationFunctionType

    nc = bacc.Bacc(
        "TRN2", target_bir_lowering=False, debug=False,
        enable_asserts=False, num_devices=ncores, num_swdge_queues=4)

    # ---- kernel I/O ----
    x0t_d = nc.dram_tensor("x0t", [IN, NS_PAD], f16, kind="ExternalInput").ap()
    idx_d = nc.dram_tensor("idx", [128, 2 * G * 128], i16,
                           kind="ExternalInput").ap()
    sel_d = nc.dram_tensor("sel", [128, G * DEG], u8,
                           kind="ExternalInput").ap()
    w1_d = nc.dram_tensor("w1", [IN, HH + 2 * HEAD], f16,
                          kind="ExternalInput").ap()
    wh_d = nc.dram_tensor("wh", [HH, HH + 2], f16, kind="ExternalInput").ap()
    w2_d = nc.dram_tensor("w2", [HH, OUT + 2], f16,
                          kind="ExternalInput").ap()
    b1_d = nc.dram_tensor("b1", [128, HH], f32, kind="ExternalInput").ap()
    bh_d = nc.dram_tensor("bh", [128, HH], f32, kind="ExternalInput").ap()
    b2_d = nc.dram_tensor("b2", [128, OUT], f32, kind="ExternalInput").ap()
    out_d = nc.dram_tensor("out", [NS_PAD, OUT], f32,
                           kind="ExternalOutput").ap()

    shared = "Shared" if ncores > 4 else "Local"
    PCB = {1: [g * 64 for g in CHB1], 2: [g * 64 for g in CHB23],
           3: [g * 64 for g in CHB23]}
    gs_t = {}
    for l, blk in ((1, BLK12), (2, BLK12), (3, BLK3)):
        pcb = PCB[l]
        gs_t[l] = [nc.dram_tensor(f"gs{l}_{c}",
                                  [pcb[c + 1] - pcb[c], blk], f16).ap()
                   for c in range(len(pcb) - 1)]
    gf1_d = nc.dram_tensor("gf1", [NBLK, BLK12], f16, addr_space=shared).ap()
    gf2_d = nc.dram_tensor("gf2", [NBLK, BLK12], f16, addr_space=shared).ap()
    gf3_d = nc.dram_tensor("gf3", [NBLK, BLK3], f16, addr_space=shared).ap()

    rgroups = [list(range(ncores))]

    with tile.TileContext(nc) as tc:
        with (
            tc.tile_pool(name="const", bufs=1) as cp,
            tc.tile_pool(name="feat", bufs=3) as fp,
            tc.tile_pool(name="edge", bufs=3) as ep,
            tc.tile_pool(name="psum", bufs=2, space="PSUM") as pp,
        ):
            nc.gpsimd.load_library(mlp)
            ident = cp.tile([128, 128], f16)
            make_identity(nc, ident[:])
            idx_sb = cp.tile([128, 2 * G * 128], i16)
            nc.sync.dma_start(out=idx_sb[:], in_=idx_d[:, :])
            sel_sb = cp.tile([128, G * DEG], u8)
            nc.sync.dma_start(out=sel_sb[:], in_=sel_d[:, :])
            x0a = cp.tile([128, NS_PAD], f16)
            x0b = cp.tile([128, NS_PAD], f16)
            nc.sync.dma_start(out=x0a[:], in_=x0t_d[0:128, :])
            nc.sync.dma_start(out=x0b[:], in_=x0t_d[128:256, :])
            w1a = cp.tile([128, HH + 2 * HEAD], f16)
            w1b = cp.tile([128, HH + 2 * HEAD], f16)
            nc.sync.dma_start(out=w1a[:], in_=w1_d[0:128, :])
            nc.sync.dma_start(out=w1b[:], in_=w1_d[128:256, :])
            wh_sb = cp.tile([128, HH + 2], f16)
            nc.sync.dma_start(out=wh_sb[:], in_=wh_d[:, :])
            w2_sb = cp.tile([128, OUT + 2], f16)
            nc.sync.dma_start(out=w2_sb[:], in_=w2_d[:, :])
            b1_t = cp.tile([128, HH], f32)
            nc.sync.dma_start(out=b1_t[:], in_=b1_d[:, :])
            bh_t = cp.tile([128, HH], f32)
            nc.sync.dma_start(out=bh_t[:], in_=bh_d[:, :])
            b2_t = cp.tile([128, OUT], f32)
            nc.sync.dma_start(out=b2_t[:], in_=b2_d[:, :])
            er1 = cp.tile([128, G * HEAD], f32)
            er2 = cp.tile([128, G], f32)
            er3 = cp.tile([128, G], f32)
            hT1 = cp.tile([128, NS_PAD], f16)
            hT2 = cp.tile([128, NS_PAD], f16)

            def feat_group(lname, g, lhsT_tiles, w_tiles, HD, H, er_t,
                           l, fp8feat, subc):
                s = slice(g * 128, (g + 1) * 128)
                nw = HD + 2 * H
                fps = pp.tile([128, HH + 2 * HEAD], f32, tag="fps",
                              name=f"{lname}_fps{g}")
                nchunk = len(lhsT_tiles)
                for c in range(nchunk):
                    nc.tensor.matmul(
                        fps[:, 0:nw], lhsT=lhsT_tiles[c][:, s],
                        rhs=w_tiles[c][:],
                        start=(c == 0), stop=(c == nchunk - 1))
                grow = fp.tile([128, subc], f16, tag=f"grow{l}",
                               name=f"{lname}_grow{g}")
                if fp8feat:
                    nc.scalar.activation(
                        out=grow[:, 0:FD].bitcast(f8), in_=fps[:, 0:HD],
                        func=AF.Copy)
                    nc.scalar.activation(
                        out=grow[:, FD:FD + H], in_=fps[:, HD:HD + H],
                        func=AF.Copy)
                else:
                    nc.scalar.activation(out=grow[:], in_=fps[:, 0:subc],
                                         func=AF.Copy)
                nc.scalar.activation(out=er_t[:, g * H:(g + 1) * H],
                                     in_=fps[:, HD + H:HD + 2 * H],
                                     func=AF.Copy)
                pcb = PCB[l]
                chb = CHB1 if l == 1 else CHB23
                c = 0
                while g >= chb[c + 1]:
                    c += 1
                p0 = g * 64 - pcb[c]
                dst = gs_t[l][c][p0:p0 + 64, 0:2 * subc].rearrange(
                    "b (s c) -> b s c", c=subc)
                nc.sync.dma_start(out=dst, in_=grow[:])

            def ag_chunk(l, c, gf_ap):
                pcb = PCB[l]
                nc.gpsimd.collective_compute(
                    "AllGather", OPT.bypass, replica_groups=rgroups,
                    ins=[gs_t[l][c][:, :]],
                    outs=[gf_ap[ncores * pcb[c]:ncores * pcb[c + 1], :]])

            def emit_gathers(lname, g, gf_ap, blk, subc, bigtag, bigw,
                             idx_off):
                ELEM = 2 * subc
                big = ep.tile([128, bigw], f16, tag=bigtag, bufs=6,
                              name=f"{lname}_big{g}")
                for h in range(2):
                    _dma_gather_raw(
                        nc, mybir,
                        big[:, h * 8 * ELEM:(h + 1) * 8 * ELEM],
                        gf_ap[:, 0:ELEM],
                        idx_sb[:, idx_off + g * 128 + h * 64:
                               idx_off + g * 128 + (h + 1) * 64],
                        NIDX, ELEM, blk, queue_num=(2 * g + h) % 4)
                return big

            def edge_math(lname, g, big, subc, HD, H, b_t, er_t, mode,
                          hT_out, fp8feat):
                D = HD // H
                ELEM = 2 * subc
                bv = big[:, 0:DEG * ELEM].rearrange("p (k r) -> p k r",
                                                    r=ELEM)
                lo = bv[:, :, 0:subc]
                hi = bv[:, :, subc:2 * subc]
                mask = (sel_sb[:, g * DEG:(g + 1) * DEG]
                        .unsqueeze(2).to_broadcast((128, DEG, subc)))
                nc.vector.copy_predicated(out=lo, mask=mask, data=hi)
                if fp8feat:
                    feat_e = bv[:, :, 0:FD].bitcast(f8)
                    el_e = bv[:, :, FD:FD + H]
                else:
                    feat_e = bv[:, :, 0:HD]
                    el_e = bv[:, :, HD:HD + H]
                # e = el + er  (er broadcast along slots)
                e_t = ep.tile([128, DEG * H], f32, tag="e_t",
                              name=f"{lname}_et{g}")
                etv = e_t[:].rearrange("p (k h) -> p k h", h=H)
                erv = (er_t[:, g * H:(g + 1) * H]
                       .unsqueeze(1).to_broadcast((128, DEG, H)))
                nc.vector.tensor_tensor(out=etv, in0=el_e, in1=erv,
                                        op=OPT.add)
                e2 = ep.tile([128, DEG * H], f32, tag="e2",
                             name=f"{lname}_e2{g}")
                nc.vector.scalar_tensor_tensor(
                    out=e2[:], in0=e_t[:], scalar=NEG_SLOPE, in1=e_t[:],
                    op0=OPT.mult, op1=OPT.max)
                ex16 = ep.tile([128, DEG * H], f16, tag="ex16",
                               name=f"{lname}_ex16{g}")
                nc.scalar.activation(out=ex16[:], in_=e2[:], func=AF.Exp)
                den = ep.tile([128, H], f32, tag="den",
                              name=f"{lname}_den{g}")
                nc.vector.tensor_reduce(
                    out=den[:],
                    in_=ex16[:].rearrange("p (k h) -> p h k", h=H),
                    axis=AX.X, op=OPT.add)
                inv = ep.tile([128, H], f32, tag="inv",
                              name=f"{lname}_inv{g}")
                nc.vector.reciprocal(inv[:], den[:])
                # alpha-weighted sum: multiply + level-1 add on DVE,
                # remaining 8 slots accumulated on PE via identity matmuls
                f_all = ep.tile([128, DEG * HD], f16, tag=f"fa{HD}",
                                name=f"{lname}_fa{g}")
                if H == 1:
                    exv = (ex16[:].rearrange("p (k h) -> p k h", h=1)
                           .to_broadcast((128, DEG, HD)))
                    nc.vector.tensor_tensor(
                        out=f_all[:].rearrange("p (k d) -> p k d", k=DEG),
                        in0=feat_e, in1=exv, op=OPT.mult)
                else:
                    featv = feat_e.rearrange("p k (h d) -> p k h d", h=H)
                    exv = (ex16[:].rearrange("p (k h) -> p k h", h=H)
                           .unsqueeze(3).to_broadcast((128, DEG, H, D)))
                    nc.vector.tensor_tensor(
                        out=f_all[:].rearrange("p (k h d) -> p k h d",
                                               k=DEG, h=H),
                        in0=featv, in1=exv, op=OPT.mult)
                u8t = ep.tile([128, 8 * HD], f16, tag=f"u{HD}",
                              name=f"{lname}_u{g}")
                nc.vector.tensor_tensor(
                    out=u8t[:], in0=f_all[:, 0:8 * HD],
                    in1=f_all[:, 8 * HD:16 * HD], op=OPT.add)
                ups = pp.tile([128, HD], f32, tag="ups",
                              name=f"{lname}_ups{g}")
                for k in range(8):
                    nc.tensor.matmul(
                        ups[:], lhsT=ident[:],
                        rhs=u8t[:, k * HD:(k + 1) * HD],
                        start=(k == 0), stop=(k == 7))
                ht = ep.tile([128, HD], f32, tag="ht",
                             name=f"{lname}_ht{g}")
                if H == 1:
                    nc.vector.scalar_tensor_tensor(
                        out=ht[:], in0=ups[:, 0:HD], scalar=inv[:, 0:1],
                        in1=b_t[:, 0:HD], op0=OPT.mult, op1=OPT.add)
                else:
                    t1 = ep.tile([128, HD], f32, tag="t1",
                                 name=f"{lname}_t1{g}")
                    invv = inv[:].unsqueeze(2).to_broadcast((128, H, D))
                    nc.vector.tensor_tensor(
                        out=t1[:].rearrange("p (h d) -> p h d", h=H),
                        in0=ups[:, 0:HD].rearrange("p (h d) -> p h d",
                                                   h=H),
                        in1=invv, op=OPT.mult)
                    nc.vector.tensor_tensor(
                        out=ht[:], in0=t1[:], in1=b_t[:, 0:HD], op=OPT.add)
                if mode == "relu":
                    hrelu = ep.tile([128, HD], f16, tag="hr",
                                    name=f"{lname}_hr{g}")
                    nc.vector.scalar_tensor_tensor(
                        out=hrelu[:], in0=ht[:], scalar=0.0, in1=ht[:],
                        op0=OPT.max, op1=OPT.max)
                    trp = pp.tile([128, 128], f16, tag="trp",
                                  name=f"{lname}_trp{g}")
                    nc.tensor.transpose(trp[:], hrelu[:], ident[:])
                    nc.scalar.activation(
                        out=hT_out[:, g * 128:(g + 1) * 128], in_=trp[:],
                        func=AF.Copy)
                else:  # logsoftmax (final layer)
                    r0, r1 = g * 128, (g + 1) * 128
                    nm_t = ep.tile([128, 1], f32, tag="nm",
                                   name=f"{lname}_nm{g}")
                    nc.vector.reduce_max(out=nm_t[:], in_=ht[:],
                                         axis=AX.X, negate=True)
                    exf = ep.tile([128, HD], f32, tag="exf",
                                  name=f"{lname}_exf{g}")
                    s_t = ep.tile([128, 1], f32, tag="s_t",
                                  name=f"{lname}_s{g}")
                    nc.scalar.activation(out=exf[:], in_=ht[:],
                                         func=AF.Exp, bias=nm_t[:],
                                         accum_out=s_t[:])
                    ls = ep.tile([128, 1], f32, tag="ls",
                                 name=f"{lname}_ls{g}")
                    nc.scalar.activation(out=ls[:], in_=s_t[:], func=AF.Ln)
                    o_t = ep.tile([128, HD], f32, tag="o_t",
                                  name=f"{lname}_o{g}")
                    nc.vector.scalar_tensor_tensor(
                        out=o_t[:], in0=ht[:], scalar=nm_t[:],
                        in1=ls[:].to_broadcast((128, HD)),
                        op0=OPT.add, op1=OPT.subtract)
                    nc.sync.dma_start(out=out_d[r0:r1, :], in_=o_t[:])

            def edge_loop(lname, gf_ap, blk, subc, HD, H, b_t, er_t, mode,
                          hT_out, idx_off, fp8feat, next_feat=None,
                          next_ag=None):
                bigtag = "big12" if blk == BLK12 else "big3"
                bigw = DEG * 2 * (SUBC1 if blk == BLK12 else SUB3)
                bigs = {}
                for gi in range(G + DEFER):
                    if gi < G:
                        bigs[gi] = emit_gathers(lname, gi, gf_ap, blk,
                                                subc, bigtag, bigw,
                                                idx_off)
                    g = gi - DEFER
                    if g >= 0:
                        edge_math(lname, g, bigs.pop(g), subc, HD, H, b_t,
                                  er_t, mode, hT_out, fp8feat)
                        if next_feat is not None:
                            next_feat(g)
                        if next_ag is not None:
                            for c in range(len(CHB23) - 1):
                                if g == CHB23[c + 1] - 1:
                                    next_ag(c)

            # ---- layer 1 feat + single-shot AG1 ----
            for g in range(G):
                feat_group("L1", g, [x0a, x0b], [w1a, w1b], HH, HEAD, er1,
                           1, True, SUBC1)
            ag_chunk(1, 0, gf1_d)
            # ---- layer 1 edge (+ layer 2 feat + AG2 interleaved) ----
            edge_loop("L1", gf1_d, BLK12, SUBC1, HH, HEAD, b1_t, er1,
                      "relu", hT1, 0, True,
                      next_feat=lambda g: feat_group(
                          "L2", g, [hT1], [wh_sb], HH, 1, er2, 2, True,
                          SUBC2),
                      next_ag=lambda c: ag_chunk(2, c, gf2_d))
            # ---- layer 2 edge (+ layer 3 feat + AG3 interleaved) ----
            edge_loop("L2", gf2_d, BLK12, SUBC2, HH, 1, bh_t, er2,
                      "relu", hT2, G * 128, True,
                      next_feat=lambda g: feat_group(
                          "L3", g, [hT2], [w2_sb], OUT, 1, er3, 3, False,
                          SUB3),
                      next_ag=lambda c: ag_chunk(3, c, gf3_d))
            # ---- layer 3 edge ----
            edge_loop("L3", gf3_d, BLK3, SUB3, OUT, 1, b2_t, er3,
                      "logsoftmax", None, G * 128, False)

    nc.compile()
    return nc


# ========================================================================
# host side
# ========================================================================
def _get_program(ncores):
    if ncores not in _PROGRAM_CACHE:
        _PROGRAM_CACHE[ncores] = _build_program(ncores)
    return _PROGRAM_CACHE[ncores]


def _numpy_fallback(feats, src, dst, W1, al1, ar1, b1, Wh, alh, arh, bh,
                    W2, al2, ar2, b2):
    n = feats.shape[0]

    def gat(x, W, al, ar, b):
        Hh, Dd = al.shape
        feat = (x @ W).reshape(n, Hh, Dd)
        el = (feat * al).sum(-1)
        er = (feat * ar).sum(-1)
        e = el[src] + er[dst]
        e = np.where(e > 0, e, NEG_SLOPE * e).astype(np.float32)
        emax = np.full((n, Hh), -np.inf, np.float32)
        np.maximum.at(emax, dst, e)
        ex = np.exp(e - emax[dst])
        den = np.zeros((n, Hh), np.float32)
        np.add.at(den, dst, ex)
        alpha = ex / den[dst]
        out = np.zeros((n, Hh, Dd), np.float32)
        np.add.at(out, dst, feat[src] * alpha[..., None])
        return out + b.reshape(1, Hh, Dd)

    h = np.maximum(gat(feats, W1, al1, ar1, b1).reshape(n, HH), 0.0)
    h = np.maximum(gat(h, Wh, alh, arh, bh).mean(1), 0.0)
    h = gat(h, W2, al2, ar2, b2).mean(1)
    m = h.max(1, keepdims=True)
    ls = np.log(np.exp(h - m).sum(1, keepdims=True))
    return (h - m - ls).astype(np.float32)


def _pair_rows(src, chb):
    """Global pair-row id for each edge source under the given AllGather
    chunk layout (chunk boundaries in groups)."""
    r = src // NV
    i = src % NV
    j = i // 2
    pch = np.asarray([g * 64 for g in chb])
    pc = pch[1:] - pch[:-1]
    gb = NCORES * pch
    c = np.searchsorted(pch[1:-1], j, side="right")
    prow = gb[c] + r * pc[c] + (j - pch[c])
    return prow.astype(np.int16)


def _idx_table(prow_core):
    """[128, G*128] int16 gather-index tile from per-edge pair rows."""
    epad = np.zeros(NS_PAD * DEG, np.int16)
    epad[:NV * DEG] = prow_core
    ev = epad.reshape(G, 128, DEG)               # [g, p, k]
    idx = np.empty((128, G * 128), np.int16)
    for g in range(G):
        for h in range(2):
            lst = ev[g, :, 8 * h:8 * h + 8].T.reshape(-1)  # i = j*128 + p
            a = lst.reshape(64, 16).T            # [16, 64]
            idx[:, g * 128 + h * 64:g * 128 + (h + 1) * 64] = np.tile(
                a, (8, 1))
    return idx


def kernel(**inputs) -> np.ndarray:
    global LAST_RESULTS
    feats = np.ascontiguousarray(np.asarray(inputs["features"],
                                            dtype=np.float32))
    src = np.asarray(inputs["src"]).astype(np.int64).ravel()
    dst = np.asarray(inputs["dst"]).astype(np.int64).ravel()
    W1 = np.asarray(inputs["W1"], dtype=np.float32)
    al1 = np.asarray(inputs["al1"], dtype=np.float32)
    ar1 = np.asarray(inputs["ar1"], dtype=np.float32)
    b1 = np.asarray(inputs["b1"], dtype=np.float32)
    Wh = np.asarray(inputs["Wh"], dtype=np.float32)
    alh = np.asarray(inputs["alh"], dtype=np.float32)
    arh = np.asarray(inputs["arh"], dtype=np.float32)
    bh = np.asarray(inputs["bh"], dtype=np.float32)
    W2 = np.asarray(inputs["W2"], dtype=np.float32)
    al2 = np.asarray(inputs["al2"], dtype=np.float32)
    ar2 = np.asarray(inputs["ar2"], dtype=np.float32)
    b2 = np.asarray(inputs["b2"], dtype=np.float32)

    n = feats.shape[0]
    expected_dst = np.repeat(np.arange(N, dtype=np.int64), DEG)
    if (n != N or src.shape[0] != N * DEG
            or not np.array_equal(dst, expected_dst)
            or src.min() < 0 or src.max() >= N):
        return _numpy_fallback(feats, src, dst, W1, al1, ar1, b1,
                               Wh, alh, arh, bh, W2, al2, ar2, b2)

    from concourse.bass_utils import run_bass_kernel_spmd

    nc = _get_program(NCORES)
    prow1 = _pair_rows(src, CHB1)
    prow23 = _pair_rows(src, CHB23)
    q = (src % NV % 2).astype(np.uint8)

    def bcast(a, w):
        return np.ascontiguousarray(
            np.broadcast_to(a.reshape(1, w), (128, w)).astype(np.float32))

    def ext16(W, al, ar):
        Hh, Dd = al.shape
        Wr = W.reshape(W.shape[0], Hh, Dd)
        wal = np.einsum("khd,hd->kh", Wr, al)
        war = np.einsum("khd,hd->kh", Wr, ar)
        return np.ascontiguousarray(
            np.concatenate([W, wal, war], axis=1).astype(np.float16))

    common = dict(
        w1=ext16(W1, al1, ar1), wh=ext16(Wh, alh, arh),
        w2=ext16(W2, al2, ar2),
        b1=bcast(b1, HH), bh=bcast(bh, HH), b2=bcast(b2, OUT),
    )
    in_maps = []
    for r in range(NCORES):
        x0t = np.zeros((IN, NS_PAD), np.float16)
        x0t[:, :NV] = feats[r * NV:(r + 1) * NV].T.astype(np.float16)
        lo, hi = r * NV * DEG, (r + 1) * NV * DEG
        idx = np.concatenate([_idx_table(prow1[lo:hi]),
                              _idx_table(prow23[lo:hi])], axis=1)
        eqpad = np.zeros(NS_PAD * DEG, np.uint8)
        eqpad[:NV * DEG] = q[lo:hi]
        sel = np.ascontiguousarray(
            eqpad.reshape(G, 128, DEG).transpose(1, 0, 2).reshape(128, -1))
        in_maps.append(dict(x0t=x0t, idx=idx, sel=sel, **common))

    trace = bool(int(os.environ.get("GAT_TRACE", "0")))
    LAST_RESULTS = run_bass_kernel_spmd(
        nc, in_maps, list(range(NCORES)), trace=trace)
    outs = [LAST_RESULTS.results[r]["out"][:NV] for r in range(NCORES)]
    return np.ascontiguousarray(np.concatenate(outs, axis=0),
                                dtype=np.float32)


# revision 15
# speedup vs baseline: 7.8049x; 1.0142x over previous
"""Trainium2 Bass kernel: 3-layer GAT (nn_GAT_62182536511748).

Strategy (8 NeuronCores, SPMD, pair-block gather, v6):
  - Nodes sharded contiguously across cores (6250 valid/core, padded to
    6272 = 49*128). dst == repeat(arange(N), 16): 16 in-edges per node.
  - Per layer each core computes feat = x_shard @ Wext (fp16 PE, PSUM
    fp32) where Wext = [W | W@al | W@ar] also yields el/er. Rows
    [feat|el] are packed into PAIR blocks (fp8 e4m3 feat + fp16 el for
    layers 1-2, fp16 for layer 3) and AllGather'd: layer 1 in one shot
    (head), layers 2-3 in 5 row chunks triggered from inside the
    previous layer's gather stream so the CC transfers overlap the
    gather drain.
  - Edge phase: per 128-node group (2048 edges), two 1024-index
    dma_gather instructions (int16 pair indices = src//2, rotating over
    the 4 SWDGE queues) fetch one pair block per edge. The gather drain
    (~440 descriptors/us, descriptor-count-bound) is the wall; the rest
    is spread across engines to hide underneath it:
      DVE: pair-select (copy_predicated), attention logits, softmax
           denominator, alpha multiply + first reduction level,
           normalize + relu.
      Scalar: exp (fp16 out), PSUM->SBUF packing copies.
      PE: remaining slot reduction as identity-matmul PSUM accumulation,
          feat matmuls, output transpose into the SBUF-resident hT tile
          that feeds the next layer (no DRAM round trip).
  - Edge math for group g is emitted three groups behind its gathers so
    every engine stream has slack; softmax skips max-subtraction
    (logits are O(1)).
"""

import os
import numpy as np

# ---- fixed problem dims -------------------------------------------------
N = 50000
DEG = 16
IN = 256
HID = 32
HEAD = 4
OUT = 40
HH = HID * HEAD  # 128
NEG_SLOPE = 0.2
NCORES = 8
NV = N // NCORES          # 6250 valid nodes per core
G = 49                    # groups of 128 rows
NS_PAD = G * 128          # 6272
NSB = NS_PAD // 2         # 3136 local pair rows
NBLK = NCORES * NSB       # 25088 global pair rows

# AllGather row chunks (in groups): L1 two chunks (overlap the feat tail),
# L2/L3 six chunks with small last chunks to shrink the layer-boundary
# bubble (they trigger only after the previous layer's gathers finish).
CHB1 = [0, 25, 49]
CHB23 = [0, 12, 24, 34, 44, 47, 49]

FD = 64                    # fp8 feat payload in f16-container elems
SUBC1, SUBC2 = FD + HEAD, FD + 1   # 68, 65 container elems per half
SUB3 = OUT + 1             # 41 (fp16)
BLK12, BLK3 = 256, 128     # table pitch (f16 elems): 512B / 256B
NIDX = 1024
DEFER = 3

_PROGRAM_CACHE = {}
LAST_RESULTS = None


def _dma_gather_raw(nc, mybir, out_ap, in_ap, idxs_ap, num_idxs, elem_size,
                    elem_step, queue_num=0):
    """dma_gather minus the over-strict elem%256B assert (stride must still
    be a 256B multiple; verified on HW with 528B/272B/164B elems)."""
    eng = nc.gpsimd
    stride_bytes = elem_step * mybir.dt.size(in_ap.dtype)
    assert stride_bytes % 256 == 0 and stride_bytes // 256 < 256
    _in_ap = eng.lower_ap_dma(in_ap, for_custom_bir_dma=True)
    _idxs_ap = eng.lower_ap(idxs_ap)
    _out_ap = eng.lower_ap(out_ap)
    return eng.add_instruction(
        mybir.InstDMAGatherAnt(
            name=nc.get_next_instruction_name(),
            ins=[*_in_ap, _idxs_ap,
                 eng.lower_val_access(eng.to_reg(num_idxs))],
            outs=[_out_ap],
            transpose=False, num_idxs=num_idxs, elem_size=elem_size,
            stride_bytes_256=stride_bytes // 256, gen_mode=0,
            single_packet=True, queue_num=queue_num,
            sbuf_tokens_per_rank=0, sbuf_free_dim_per_rank=0,
            sbuf_free_dim_pad_per_rank=0, sbuf_byte_offset=0,
        ))


# ========================================================================
# device program
# ========================================================================
def _build_program(ncores: int):
    from concourse import bass, mybir, tile, bacc
    from concourse.masks import make_identity
    from concourse.library_config import mlp

    f32 = mybir.dt.float32
    f16 = mybir.dt.float16
    f8 = mybir.dt.float8e4
    i16 = mybir.dt.int16
    u8 = mybir.dt.uint8
    AX = mybir.AxisListType
    OPT = mybir.AluOpType
    AF = mybir.ActivationFunctionType

    nc = bacc.Bacc(
        "TRN2", target_bir_lowering=False, debug=False,
        enable_asserts=False, num_devices=ncores, num_swdge_queues=4)

    # ---- kernel I/O ----
    x0t_d = nc.dram_tensor("x0t", [IN, NS_PAD], f16, kind="ExternalInput").ap()
    idx_d = nc.dram_tensor("idx", [128, 2 * G * 128], i16,
                           kind="ExternalInput").ap()
    sel_d = nc.dram_tensor("sel", [128, G * DEG], u8,
                           kind="ExternalInput").ap()
    w1_d = nc.dram_tensor("w1", [IN, HH + 2 * HEAD], f16,
                          kind="ExternalInput").ap()
    wh_d = nc.dram_tensor("wh", [HH, HH + 2], f16, kind="ExternalInput").ap()
    w2_d = nc.dram_tensor("w2", [HH, OUT + 2], f16,
                          kind="ExternalInput").ap()
    b1_d = nc.dram_tensor("b1", [128, HH], f32, kind="ExternalInput").ap()
    bh_d = nc.dram_tensor("bh", [128, HH], f32, kind="ExternalInput").ap()
    b2_d = nc.dram_tensor("b2", [128, OUT], f32, kind="ExternalInput").ap()
    out_d = nc.dram_tensor("out", [NS_PAD, OUT], f32,
                           kind="ExternalOutput").ap()

    shared = "Shared" if ncores > 4 else "Local"
    PCB = {1: [g * 64 for g in CHB1], 2: [g * 64 for g in CHB23],
           3: [g * 64 for g in CHB23]}
    gs_t = {}
    for l, blk in ((1, BLK12), (2, BLK12), (3, BLK3)):
        pcb = PCB[l]
        gs_t[l] = [nc.dram_tensor(f"gs{l}_{c}",
                                  [pcb[c + 1] - pcb[c], blk], f16).ap()
                   for c in range(len(pcb) - 1)]
    gf1_d = nc.dram_tensor("gf1", [NBLK, BLK12], f16, addr_space=shared).ap()
    gf2_d = nc.dram_tensor("gf2", [NBLK, BLK12], f16, addr_space=shared).ap()
    gf3_d = nc.dram_tensor("gf3", [NBLK, BLK3], f16, addr_space=shared).ap()

    rgroups = [list(range(ncores))]

    with tile.TileContext(nc) as tc:
        with (
            tc.tile_pool(name="const", bufs=1) as cp,
            tc.tile_pool(name="feat", bufs=3) as fp,
            tc.tile_pool(name="edge", bufs=3) as ep,
            tc.tile_pool(name="psum", bufs=2, space="PSUM") as pp,
        ):
            nc.gpsimd.load_library(mlp)
            ident = cp.tile([128, 128], f16)
            make_identity(nc, ident[:])
            idx_sb = cp.tile([128, 2 * G * 128], i16)
            nc.sync.dma_start(out=idx_sb[:], in_=idx_d[:, :])
            sel_sb = cp.tile([128, G * DEG], u8)
            nc.sync.dma_start(out=sel_sb[:], in_=sel_d[:, :])
            x0a = cp.tile([128, NS_PAD], f16)
            x0b = cp.tile([128, NS_PAD], f16)
            nc.sync.dma_start(out=x0a[:], in_=x0t_d[0:128, :])
            nc.sync.dma_start(out=x0b[:], in_=x0t_d[128:256, :])
            w1a = cp.tile([128, HH + 2 * HEAD], f16)
            w1b = cp.tile([128, HH + 2 * HEAD], f16)
            nc.sync.dma_start(out=w1a[:], in_=w1_d[0:128, :])
            nc.sync.dma_start(out=w1b[:], in_=w1_d[128:256, :])
            wh_sb = cp.tile([128, HH + 2], f16)
            nc.sync.dma_start(out=wh_sb[:], in_=wh_d[:, :])
            w2_sb = cp.tile([128, OUT + 2], f16)
            nc.sync.dma_start(out=w2_sb[:], in_=w2_d[:, :])
            b1_t = cp.tile([128, HH], f32)
            nc.sync.dma_start(out=b1_t[:], in_=b1_d[:, :])
            bh_t = cp.tile([128, HH], f32)
            nc.sync.dma_start(out=bh_t[:], in_=bh_d[:, :])
            b2_t = cp.tile([128, OUT], f32)
            nc.sync.dma_start(out=b2_t[:], in_=b2_d[:, :])
            er1 = cp.tile([128, G * HEAD], f32)
            er2 = cp.tile([128, G], f32)
            er3 = cp.tile([128, G], f32)
            hT1 = cp.tile([128, NS_PAD], f16)
            hT2 = cp.tile([128, NS_PAD], f16)

            def feat_group(lname, g, lhsT_tiles, w_tiles, HD, H, er_t,
                           l, fp8feat, subc):
                s = slice(g * 128, (g + 1) * 128)
                nw = HD + 2 * H
                fps = pp.tile([128, HH + 2 * HEAD], f32, tag="fps",
                              name=f"{lname}_fps{g}")
                nchunk = len(lhsT_tiles)
                for c in range(nchunk):
                    nc.tensor.matmul(
                        fps[:, 0:nw], lhsT=lhsT_tiles[c][:, s],
                        rhs=w_tiles[c][:],
                        start=(c == 0), stop=(c == nchunk - 1))
                grow = fp.tile([128, subc], f16, tag=f"grow{l}",
                               name=f"{lname}_grow{g}")
                if l == 1:
                    # head phase: DVE is idle, scalar would serialize
                    nc.vector.tensor_copy(grow[:, 0:FD].bitcast(f8),
                                          fps[:, 0:HD])
                    nc.vector.tensor_copy(grow[:, FD:FD + H],
                                          fps[:, HD:HD + H])
                    nc.vector.tensor_copy(er_t[:, g * H:(g + 1) * H],
                                          fps[:, HD + H:HD + 2 * H])
                    grow_done = None
                else:
                    if fp8feat:
                        nc.scalar.activation(
                            out=grow[:, 0:FD].bitcast(f8), in_=fps[:, 0:HD],
                            func=AF.Copy)
                        nc.scalar.activation(
                            out=grow[:, FD:FD + H], in_=fps[:, HD:HD + H],
                            func=AF.Copy)
                    else:
                        nc.scalar.activation(out=grow[:],
                                             in_=fps[:, 0:subc],
                                             func=AF.Copy)
                    nc.scalar.activation(out=er_t[:, g * H:(g + 1) * H],
                                         in_=fps[:, HD + H:HD + 2 * H],
                                         func=AF.Copy)
                pcb = PCB[l]
                chb = CHB1 if l == 1 else CHB23
                c = 0
                while g >= chb[c + 1]:
                    c += 1
                p0 = g * 64 - pcb[c]
                dst = gs_t[l][c][p0:p0 + 64, 0:2 * subc].rearrange(
                    "b (s c) -> b s c", c=subc)
                nc.sync.dma_start(out=dst, in_=grow[:])

            def ag_chunk(l, c, gf_ap):
                pcb = PCB[l]
                nc.gpsimd.collective_compute(
                    "AllGather", OPT.bypass, replica_groups=rgroups,
                    ins=[gs_t[l][c][:, :]],
                    outs=[gf_ap[ncores * pcb[c]:ncores * pcb[c + 1], :]])

            def emit_gathers(lname, g, gf_ap, blk, subc, bigtag, bigw,
                             idx_off):
                ELEM = 2 * subc
                big = ep.tile([128, bigw], f16, tag=bigtag, bufs=6,
                              name=f"{lname}_big{g}")
                for h in range(2):
                    _dma_gather_raw(
                        nc, mybir,
                        big[:, h * 8 * ELEM:(h + 1) * 8 * ELEM],
                        gf_ap[:, 0:ELEM],
                        idx_sb[:, idx_off + g * 128 + h * 64:
                               idx_off + g * 128 + (h + 1) * 64],
                        NIDX, ELEM, blk, queue_num=(2 * g + h) % 4)
                return big

            def edge_math(lname, g, big, subc, HD, H, b_t, er_t, mode,
                          hT_out, fp8feat):
                D = HD // H
                ELEM = 2 * subc
                bv = big[:, 0:DEG * ELEM].rearrange("p (k r) -> p k r",
                                                    r=ELEM)
                lo = bv[:, :, 0:subc]
                hi = bv[:, :, subc:2 * subc]
                mask = (sel_sb[:, g * DEG:(g + 1) * DEG]
                        .unsqueeze(2).to_broadcast((128, DEG, subc)))
                nc.vector.copy_predicated(out=lo, mask=mask, data=hi)
                if fp8feat:
                    feat_e = bv[:, :, 0:FD].bitcast(f8)
                    el_e = bv[:, :, FD:FD + H]
                else:
                    feat_e = bv[:, :, 0:HD]
                    el_e = bv[:, :, HD:HD + H]
                # e = el + er  (er broadcast along slots)
                e_t = ep.tile([128, DEG * H], f32, tag="e_t",
                              name=f"{lname}_et{g}")
                etv = e_t[:].rearrange("p (k h) -> p k h", h=H)
                erv = (er_t[:, g * H:(g + 1) * H]
                       .unsqueeze(1).to_broadcast((128, DEG, H)))
                nc.vector.tensor_tensor(out=etv, in0=el_e, in1=erv,
                                        op=OPT.add)
                e2 = ep.tile([128, DEG * H], f32, tag="e2",
                             name=f"{lname}_e2{g}")
                nc.vector.scalar_tensor_tensor(
                    out=e2[:], in0=e_t[:], scalar=NEG_SLOPE, in1=e_t[:],
                    op0=OPT.mult, op1=OPT.max)
                ex16 = ep.tile([128, DEG * H], f16, tag="ex16",
                               name=f"{lname}_ex16{g}")
                nc.scalar.activation(out=ex16[:], in_=e2[:], func=AF.Exp)
                den = ep.tile([128, H], f32, tag="den",
                              name=f"{lname}_den{g}")
                nc.vector.tensor_reduce(
                    out=den[:],
                    in_=ex16[:].rearrange("p (k h) -> p h k", h=H),
                    axis=AX.X, op=OPT.add)
                inv = ep.tile([128, H], f32, tag="inv",
                              name=f"{lname}_inv{g}")
                nc.vector.reciprocal(inv[:], den[:])
                # alpha-weighted sum: multiply + level-1 add on DVE,
                # remaining 8 slots accumulated on PE via identity matmuls
                f_all = ep.tile([128, DEG * HD], f16, tag=f"fa{HD}",
                                name=f"{lname}_fa{g}")
                if H == 1:
                    exv = (ex16[:].rearrange("p (k h) -> p k h", h=1)
                           .to_broadcast((128, DEG, HD)))
                    nc.vector.tensor_tensor(
                        out=f_all[:].rearrange("p (k d) -> p k d", k=DEG),
                        in0=feat_e, in1=exv, op=OPT.mult)
                else:
                    featv = feat_e.rearrange("p k (h d) -> p k h d", h=H)
                    exv = (ex16[:].rearrange("p (k h) -> p k h", h=H)
                           .unsqueeze(3).to_broadcast((128, DEG, H, D)))
                    nc.vector.tensor_tensor(
                        out=f_all[:].rearrange("p (k h d) -> p k h d",
                                               k=DEG, h=H),
                        in0=featv, in1=exv, op=OPT.mult)
                u8t = ep.tile([128, 8 * HD], f16, tag=f"u{HD}",
                              name=f"{lname}_u{g}")
                nc.vector.tensor_tensor(
                    out=u8t[:], in0=f_all[:, 0:8 * HD],
                    in1=f_all[:, 8 * HD:16 * HD], op=OPT.add)
                ups = pp.tile([128, HD], f32, tag="ups",
                              name=f"{lname}_ups{g}")
                for k in range(8):
                    nc.tensor.matmul(
                        ups[:], lhsT=ident[:],
                        rhs=u8t[:, k * HD:(k + 1) * HD],
                        start=(k == 0), stop=(k == 7))
                ht = ep.tile([128, HD], f32, tag="ht",
                             name=f"{lname}_ht{g}")
                if H == 1:
                    nc.vector.scalar_tensor_tensor(
                        out=ht[:], in0=ups[:, 0:HD], scalar=inv[:, 0:1],
                        in1=b_t[:, 0:HD], op0=OPT.mult, op1=OPT.add)
                else:
                    t1 = ep.tile([128, HD], f32, tag="t1",
                                 name=f"{lname}_t1{g}")
                    invv = inv[:].unsqueeze(2).to_broadcast((128, H, D))
                    nc.vector.tensor_tensor(
                        out=t1[:].rearrange("p (h d) -> p h d", h=H),
                        in0=ups[:, 0:HD].rearrange("p (h d) -> p h d",
                                                   h=H),
                        in1=invv, op=OPT.mult)
                    nc.vector.tensor_tensor(
                        out=ht[:], in0=t1[:], in1=b_t[:, 0:HD], op=OPT.add)
                if mode == "relu":
                    hrelu = ep.tile([128, HD], f16, tag="hr",
                                    name=f"{lname}_hr{g}")
                    nc.vector.scalar_tensor_tensor(
                        out=hrelu[:], in0=ht[:], scalar=0.0, in1=ht[:],
                        op0=OPT.max, op1=OPT.max)
                    trp = pp.tile([128, 128], f16, tag="trp",
                                  name=f"{lname}_trp{g}")
                    nc.tensor.transpose(trp[:], hrelu[:], ident[:])
                    nc.scalar.activation(
                        out=hT_out[:, g * 128:(g + 1) * 128], in_=trp[:],
                        func=AF.Copy)
                else:  # logsoftmax (final layer)
                    r0, r1 = g * 128, (g + 1) * 128
                    nm_t = ep.tile([128, 1], f32, tag="nm",
                                   name=f"{lname}_nm{g}")
                    nc.vector.reduce_max(out=nm_t[:], in_=ht[:],
                                         axis=AX.X, negate=True)
                    exf = ep.tile([128, HD], f32, tag="exf",
                                  name=f"{lname}_exf{g}")
                    s_t = ep.tile([128, 1], f32, tag="s_t",
                                  name=f"{lname}_s{g}")
                    nc.scalar.activation(out=exf[:], in_=ht[:],
                                         func=AF.Exp, bias=nm_t[:],
                                         accum_out=s_t[:])
                    ls = ep.tile([128, 1], f32, tag="ls",
                                 name=f"{lname}_ls{g}")
                    nc.scalar.activation(out=ls[:], in_=s_t[:], func=AF.Ln)
                    o_t = ep.tile([128, HD], f32, tag="o_t",
                                  name=f"{lname}_o{g}")
                    nc.vector.scalar_tensor_tensor(
                        out=o_t[:], in0=ht[:], scalar=nm_t[:],
                        in1=ls[:].to_broadcast((128, HD)),
                        op0=OPT.add, op1=OPT.subtract)
                    nc.sync.dma_start(out=out_d[r0:r1, :], in_=o_t[:])

            def edge_loop(lname, gf_ap, blk, subc, HD, H, b_t, er_t, mode,
                          hT_out, idx_off, fp8feat, next_feat=None,
                          next_ag=None):
                bigtag = "big12" if blk == BLK12 else "big3"
                bigw = DEG * 2 * (SUBC1 if blk == BLK12 else SUB3)
                bigs = {}
                for gi in range(G + DEFER):
                    if gi < G:
                        bigs[gi] = emit_gathers(lname, gi, gf_ap, blk,
                                                subc, bigtag, bigw,
                                                idx_off)
                    g = gi - DEFER
                    if g >= 0:
                        edge_math(lname, g, bigs.pop(g), subc, HD, H, b_t,
                                  er_t, mode, hT_out, fp8feat)
                        if next_feat is not None:
                            next_feat(g)
                        if next_ag is not None:
                            for c in range(len(CHB23) - 1):
                                if g == CHB23[c + 1] - 1:
                                    next_ag(c)

            # ---- layer 1 feat + AG1 (chunks interleaved with feat) ----
            for g in range(G):
                feat_group("L1", g, [x0a, x0b], [w1a, w1b], HH, HEAD, er1,
                           1, True, SUBC1)
                for c in range(len(CHB1) - 1):
                    if g == CHB1[c + 1] - 1:
                        ag_chunk(1, c, gf1_d)
            # ---- layer 1 edge (+ layer 2 feat + AG2 interleaved) ----
            edge_loop("L1", gf1_d, BLK12, SUBC1, HH, HEAD, b1_t, er1,
                      "relu", hT1, 0, True,
                      next_feat=lambda g: feat_group(
                          "L2", g, [hT1], [wh_sb], HH, 1, er2, 2, True,
                          SUBC2),
                      next_ag=lambda c: ag_chunk(2, c, gf2_d))
            # ---- layer 2 edge (+ layer 3 feat + AG3 interleaved) ----
            edge_loop("L2", gf2_d, BLK12, SUBC2, HH, 1, bh_t, er2,
                      "relu", hT2, G * 128, True,
                      next_feat=lambda g: feat_group(
                          "L3", g, [hT2], [w2_sb], OUT, 1, er3, 3, False,
                          SUB3),
                      next_ag=lambda c: ag_chunk(3, c, gf3_d))
            # ---- layer 3 edge ----
            edge_loop("L3", gf3_d, BLK3, SUB3, OUT, 1, b2_t, er3,
                      "logsoftmax", None, G * 128, False)

    nc.compile()
    return nc


# ========================================================================
# host side
# ========================================================================
def _get_program(ncores):
    if ncores not in _PROGRAM_CACHE:
        _PROGRAM_CACHE[ncores] = _build_program(ncores)
    return _PROGRAM_CACHE[ncores]


def _numpy_fallback(feats, src, dst, W1, al1, ar1, b1, Wh, alh, arh, bh,
                    W2, al2, ar2, b2):
    n = feats.shape[0]

    def gat(x, W, al, ar, b):
        Hh, Dd = al.shape
        feat = (x @ W).reshape(n, Hh, Dd)
        el = (feat * al).sum(-1)
        er = (feat * ar).sum(-1)
        e = el[src] + er[dst]
        e = np.where(e > 0, e, NEG_SLOPE * e).astype(np.float32)
        emax = np.full((n, Hh), -np.inf, np.float32)
        np.maximum.at(emax, dst, e)
        ex = np.exp(e - emax[dst])
        den = np.zeros((n, Hh), np.float32)
        np.add.at(den, dst, ex)
        alpha = ex / den[dst]
        out = np.zeros((n, Hh, Dd), np.float32)
        np.add.at(out, dst, feat[src] * alpha[..., None])
        return out + b.reshape(1, Hh, Dd)

    h = np.maximum(gat(feats, W1, al1, ar1, b1).reshape(n, HH), 0.0)
    h = np.maximum(gat(h, Wh, alh, arh, bh).mean(1), 0.0)
    h = gat(h, W2, al2, ar2, b2).mean(1)
    m = h.max(1, keepdims=True)
    ls = np.log(np.exp(h - m).sum(1, keepdims=True))
    return (h - m - ls).astype(np.float32)


def _pair_rows(src, chb):
    """Global pair-row id for each edge source under the given AllGather
    chunk layout (chunk boundaries in groups)."""
    r = src // NV
    i = src % NV
    j = i // 2
    pch = np.asarray([g * 64 for g in chb])
    pc = pch[1:] - pch[:-1]
    gb = NCORES * pch
    c = np.searchsorted(pch[1:-1], j, side="right")
    prow = gb[c] + r * pc[c] + (j - pch[c])
    return prow.astype(np.int16)


def _idx_table(prow_core):
    """[128, G*128] int16 gather-index tile from per-edge pair rows."""
    epad = np.zeros(NS_PAD * DEG, np.int16)
    epad[:NV * DEG] = prow_core
    ev = epad.reshape(G, 128, DEG)               # [g, p, k]
    idx = np.empty((128, G * 128), np.int16)
    for g in range(G):
        for h in range(2):
            lst = ev[g, :, 8 * h:8 * h + 8].T.reshape(-1)  # i = j*128 + p
            a = lst.reshape(64, 16).T            # [16, 64]
            idx[:, g * 128 + h * 64:g * 128 + (h + 1) * 64] = np.tile(
                a, (8, 1))
    return idx


def kernel(**inputs) -> np.ndarray:
    global LAST_RESULTS
    feats = np.ascontiguousarray(np.asarray(inputs["features"],
                                            dtype=np.float32))
    src = np.asarray(inputs["src"]).astype(np.int64).ravel()
    dst = np.asarray(inputs["dst"]).astype(np.int64).ravel()
    W1 = np.asarray(inputs["W1"], dtype=np.float32)
    al1 = np.asarray(inputs["al1"], dtype=np.float32)
    ar1 = np.asarray(inputs["ar1"], dtype=np.float32)
    b1 = np.asarray(inputs["b1"], dtype=np.float32)
    Wh = np.asarray(inputs["Wh"], dtype=np.float32)
    alh = np.asarray(inputs["alh"], dtype=np.float32)
    arh = np.asarray(inputs["arh"], dtype=np.float32)
    bh = np.asarray(inputs["bh"], dtype=np.float32)
    W2 = np.asarray(inputs["W2"], dtype=np.float32)
    al2 = np.asarray(inputs["al2"], dtype=np.float32)
    ar2 = np.asarray(inputs["ar2"], dtype=np.float32)
    b2 = np.asarray(inputs["b2"], dtype=np.float32)

    n = feats.shape[0]
    expected_dst = np.repeat(np.arange(N, dtype=np.int64), DEG)
    if (n != N or src.shape[0] != N * DEG
            or not np.array_equal(dst, expected_dst)
            or src.min() < 0 or src.max() >= N):
        return _numpy_fallback(feats, src, dst, W1, al1, ar1, b1,
                               Wh, alh, arh, bh, W2, al2, ar2, b2)

    from concourse.bass_utils import run_bass_kernel_spmd

    nc = _get_program(NCORES)
    prow1 = _pair_rows(src, CHB1)
    prow23 = _pair_rows(src, CHB23)
    q = (src % NV % 2).astype(np.uint8)

    def bcast(a, w):
        return np.ascontiguousarray(
            np.broadcast_to(a.reshape(1, w), (128, w)).astype(np.float32))

    def ext16(W, al, ar):
        Hh, Dd = al.shape
        Wr = W.reshape(W.shape[0], Hh, Dd)
        wal = np.einsum("khd,hd->kh", Wr, al)
        war = np.einsum("khd,hd->kh", Wr, ar)
        return np.ascontiguousarray(
            np.concatenate([W, wal, war], axis=1).astype(np.float16))

    common = dict(
        w1=ext16(W1, al1, ar1), wh=ext16(Wh, alh, arh),
        w2=ext16(W2, al2, ar2),
        b1=bcast(b1, HH), bh=bcast(bh, HH), b2=bcast(b2, OUT),
    )
    in_maps = []
    for r in range(NCORES):
        x0t = np.zeros((IN, NS_PAD), np.float16)
        x0t[:, :NV] = feats[r * NV:(r + 1) * NV].T.astype(np.float16)
        lo, hi = r * NV * DEG, (r + 1) * NV * DEG
        idx = np.concatenate([_idx_table(prow1[lo:hi]),
                              _idx_table(prow23[lo:hi])], axis=1)
        eqpad = np.zeros(NS_PAD * DEG, np.uint8)
        eqpad[:NV * DEG] = q[lo:hi]
        sel = np.ascontiguousarray(
            eqpad.reshape(G, 128, DEG).transpose(1, 0, 2).reshape(128, -1))
        in_maps.append(dict(x0t=x0t, idx=idx, sel=sel, **common))

    trace = bool(int(os.environ.get("GAT_TRACE", "0")))
    LAST_RESULTS = run_bass_kernel_spmd(
        nc, in_maps, list(range(NCORES)), trace=trace)
    outs = [LAST_RESULTS.results[r]["out"][:NV] for r in range(NCORES)]
    return np.ascontiguousarray(np.concatenate(outs, axis=0),
                                dtype=np.float32)
